# revision 1
# baseline (speedup 1.0000x reference)
"""Trainium2 Bass kernel for nn_ContrastiveLoss (segment_reduce).

Strategy (8 NeuronCores, SPMD):
  Phase 1: shard (batch r in 0..3) x (pixel-half). Each core computes the raw
    masked segment sums S_raw[q, ch] = sum_p combT[p, q] * feat[r, ch, p] for
    its 50 objects (rows i = q*4+r of the reference's N=200) over its pixel
    range, for both features_q and features_k, via PE matmuls contracting over
    pixels (fp32r). Features are transposed on-chip 128x128 via PE transpose.
  Gather: host concatenates per-core partial outputs (pure data movement).
  Phase 2: single core sums the two pixel-half partials, normalizes rows
    (the reference's /cnt cancels inside l2norm and pad), forms the 200x200
    logit matrix, and reduces to the contrastive loss scalar.
"""

import numpy as np
from contextlib import ExitStack

import concourse.bass as bass
import concourse.tile as tile
from concourse import bacc, mybir
from concourse.bass_utils import run_bass_kernel_spmd

# Problem constants (hardcoded per task spec)
B, M, C, H, W = 4, 50, 256, 100, 352
HW = H * W                  # 35200
N = B * M                   # 200
TAU = 0.07

P = 128                     # partitions / pixel tile
Q = M                       # 50 objects per batch
T = 138                     # pixel tiles per core (padded: 275 total = 138+137)
PX = T * P                  # 17664
CT = 23                     # pixel tiles per DMA chunk
NCHUNK = T // CT            # 6
F32R = mybir.dt.float32r
F32 = mybir.dt.float32
FP8 = mybir.dt.float8e4
NP_FP8 = mybir.dt.np(FP8)


# Force exp/ln to resolve to the combined "natural_log_exp_and_others" table
# set (index 6) instead of alternating single-function sets: empty the earlier
# sets we never want so first-match lands on sqrt_and_others (3) for
# sqrt/copy and natural_log_exp_and_others (6) for exp+ln. Indices are
# preserved so act_func_set_id stays aligned with act_info.json.
import concourse.bacc as _bacc_mod
import concourse.hw_specs as _hw_specs
_orig_get_tables = _hw_specs.get_activation_tables

def _patched_get_tables(module_arch):
    tables = dict(_orig_get_tables(module_arch))
    for i, k in enumerate(tables):
        if i in (0, 1, 2, 4, 5):
            tables[k] = set()
    return tables

_bacc_mod.get_activation_tables = _patched_get_tables

_cache = {}



def _build_phase1():
    nc = bacc.Bacc(None, target_bir_lowering=False, debug=False)
    with tile.TileContext(nc) as tc, ExitStack() as ctx:
        dram = ctx.enter_context(tc.tile_pool(name="dram", bufs=1, space="DRAM"))
        fq = dram.tile([C, PX], F32R, kind="ExternalInput", name="fq", uniquify=False)
        fk = dram.tile([C, PX], F32R, kind="ExternalInput", name="fk", uniquify=False)
        mat = dram.tile([P, T, Q], FP8, kind="ExternalInput", name="mat", uniquify=False)
        mbt = dram.tile([P, T, Q], FP8, kind="ExternalInput", name="mbt", uniquify=False)
        outq = dram.tile([Q, C], F32, kind="ExternalOutput", name="outq", uniquify=False)
        outk = dram.tile([Q, C], F32, kind="ExternalOutput", name="outk", uniquify=False)

        consts = ctx.enter_context(tc.tile_pool(name="consts", bufs=1))
        ident = consts.tile([P, P], F32)
        nc.gpsimd.memset(ident[:], 0.0)
        nc.gpsimd.affine_select(
            out=ident.bitcast(F32R), in_=ident.bitcast(F32R),
            compare_op=mybir.AluOpType.not_equal, fill=1.0, base=0,
            pattern=[[-1, P]], channel_multiplier=1)

        mask_pool = ctx.enter_context(tc.tile_pool(name="mask", bufs=1))
        CHUNKS = [6, 12, 16, 16, 16, 16, 16, 16, 16, 4, 4]
        assert sum(CHUNKS) == T
        C0 = CHUNKS[0]
        mat_sb0 = mask_pool.tile([P, C0, Q], FP8, name="mat_sb0")
        mbt_sb0 = mask_pool.tile([P, C0, Q], FP8, name="mbt_sb0")
        mat_sb = mask_pool.tile([P, T - C0, Q], FP8, name="mat_sb")
        mbt_sb = mask_pool.tile([P, T - C0, Q], FP8, name="mbt_sb")
        # chunk-0 masks land first (tiny), before any feature data
        nc.sync.dma_start(out=mat_sb0, in_=mat[:, 0:C0, :])
        nc.sync.dma_start(out=mbt_sb0, in_=mbt[:, 0:C0, :])

        psum_acc = ctx.enter_context(tc.tile_pool(name="psum_acc", bufs=1, space="PSUM"))
        ps = {"q": psum_acc.tile([Q, C], F32, name="ps_q"),
              "k": psum_acc.tile([Q, C], F32, name="ps_k")}

        fpools = {}
        for f in "qk":
            for cb in range(2):
                fpools[(f, cb)] = ctx.enter_context(
                    tc.tile_pool(name=f"f{f}{cb}", bufs=3))
        comb_pool = ctx.enter_context(tc.tile_pool(name="comb", bufs=4))
        featT_pool = ctx.enter_context(tc.tile_pool(name="featT", bufs=14))
        psum_t = ctx.enter_context(tc.tile_pool(name="psum_t", bufs=6, space="PSUM"))

        drams = {"q": fq, "k": fk}
        t0 = 0
        for chi, CTc in enumerate(CHUNKS):
            chunk = {}
            for f in "qk":
                for cb in range(2):
                    tl = fpools[(f, cb)].tile([P, CTc * P], F32R, name=f"f{f}{cb}t")
                    nc.sync.dma_start(
                        out=tl, in_=drams[f][cb * P:(cb + 1) * P, t0 * P:(t0 + CTc) * P])
                    chunk[(f, cb)] = tl
            if chi == 0:
                # remaining masks stream in behind the first feature chunk
                nc.sync.dma_start(out=mat_sb, in_=mat[:, C0:, :])
                nc.sync.dma_start(out=mbt_sb, in_=mbt[:, C0:, :])
            comb = comb_pool.tile([P, CTc, Q], F32R, name="comb")
            if chi == 0:
                nc.vector.tensor_mul(comb, mat_sb0, mbt_sb0)
            else:
                nc.vector.tensor_mul(comb, mat_sb[:, t0 - C0:t0 - C0 + CTc, :],
                                     mbt_sb[:, t0 - C0:t0 - C0 + CTc, :])
            for tt in range(CTc):
                t = t0 + tt
                for fi, f in enumerate("qk"):
                    ftT = featT_pool.tile([P, C], F32R, name="ftT")
                    pt = psum_t.tile([P, C], F32, name="pt")
                    for cb in range(2):
                        nc.tensor.transpose(
                            pt[:, cb * P:(cb + 1) * P].bitcast(F32R),
                            chunk[(f, cb)][:, tt * P:(tt + 1) * P],
                            ident.bitcast(F32R))
                    nc.vector.tensor_copy(ftT[:, :P], pt[:, :P].bitcast(F32R))
                    nc.scalar.copy(ftT[:, P:], pt[:, P:].bitcast(F32R))
                    nc.tensor.matmul(ps[f], comb[:, tt, :], ftT,
                                     start=(t == 0), stop=(t == T - 1))
            t0 += CTc

        out_pool = ctx.enter_context(tc.tile_pool(name="outp", bufs=1))
        for f, od in (("q", outq), ("k", outk)):
            o = out_pool.tile([Q, C], F32, name=f"o{f}")
            nc.vector.tensor_copy(o, ps[f])
            nc.sync.dma_start(out=od[:], in_=o)
    nc.compile()
    return nc


def _build_phase2():
    nc = bacc.Bacc(None, target_bir_lowering=False, debug=False)
    with tile.TileContext(nc) as tc, ExitStack() as ctx:
        dram = ctx.enter_context(tc.tile_pool(name="dram", bufs=1, space="DRAM"))
        pq = dram.tile([8, Q, C], F32, kind="ExternalInput", name="pq", uniquify=False)
        pk = dram.tile([8, Q, C], F32, kind="ExternalInput", name="pk", uniquify=False)
        out = dram.tile([1, 1], F32, kind="ExternalOutput", name="loss", uniquify=False)

        sb = ctx.enter_context(tc.tile_pool(name="sb", bufs=1))
        psum = ctx.enter_context(tc.tile_pool(name="psum", bufs=3, space="PSUM"))
        psum_nd = ctx.enter_context(tc.tile_pool(name="psum_nd", bufs=1, space="PSUM"))

        ident = sb.tile([P, P], F32)
        nc.gpsimd.memset(ident[:], 0.0)
        nc.gpsimd.affine_select(
            out=ident[:], in_=ident[:],
            compare_op=mybir.AluOpType.not_equal, fill=1.0, base=0,
            pattern=[[-1, P]], channel_multiplier=1)
        ones = sb.tile([P, P], F32)
        nc.gpsimd.memset(ones[:], 1.0)

        # Prefetch the sqrt table set during the input DMA (no data deps)
        warm = sb.tile([1, 1], F32)
        nc.scalar.sqrt(warm, ones[0:1, 0:1])

        # Load partials per (feature, batch r): (50-part, 2 halves, ch)
        raw = {}
        for nm, dt_ in (("q", pq), ("k", pk)):
            rt = sb.tile([Q, 8, C], F32, name=f"raw{nm}")
            for r in range(4):
                nc.sync.dma_start(out=rt[:, 2 * r:2 * r + 2, :],
                                  in_=dt_[2 * r:2 * r + 2].rearrange("e q c -> q e c"))
            raw[nm] = rt

        # Transpose-and-sum the two pixel-half partials directly in PSUM:
        # ST[nm][cb]: (128ch, 200) with column order i' = r*50+q
        ST = {}
        ncopy = 0
        for nm in "qk":
            for cb in range(2):
                stt = sb.tile([P, N], F32, name=f"ST{nm}{cb}")
                for r in range(4):
                    ptt = psum.tile([P, Q], F32, name="ptt", tag="ps")
                    for hf in range(2):
                        nc.tensor.matmul(
                            ptt, raw[nm][:, 2 * r + hf, cb * P:(cb + 1) * P],
                            ident[0:Q, 0:Q], is_transpose=True,
                            start=(hf == 0), stop=(hf == 1))
                    if ncopy % 2 == 0:
                        nc.vector.tensor_copy(stt[:, r * Q:(r + 1) * Q], ptt)
                    else:
                        nc.scalar.copy(stt[:, r * Q:(r + 1) * Q], ptt)
                    ncopy += 1
                ST[(nm, cb)] = stt

        # Row norms -> inv_k (scaled by 1/TAU), inv_q as (1, 200) rows
        inv = {}
        for nm in "qk":
            ps_n = psum.tile([1, N], F32, name="ps_n", tag="ps")
            for cb in range(2):
                sq_ = sb.tile([P, N], F32, name="sq_")
                nc.vector.tensor_mul(sq_, ST[(nm, cb)], ST[(nm, cb)])
                nc.tensor.matmul(ps_n, ones[:, 0:1], sq_,
                                 start=(cb == 0), stop=(cb == 1))
            nrm = sb.tile([1, N], F32, name=f"nrm{nm}")
            nc.scalar.sqrt(nrm, ps_n)
            nc.vector.tensor_scalar_max(nrm, nrm, 1e-12)
            iv = sb.tile([1, N], F32, name=f"inv{nm}")
            nc.vector.reciprocal(iv, nrm)
            inv[nm] = iv
        invk_tau = sb.tile([1, N], F32)
        nc.vector.tensor_scalar_mul(invk_tau, inv["k"], 1.0 / TAU)
        warm2 = sb.tile([1, 1], F32)
        nc.scalar.activation(warm2, inv["k"][:, 0:1],
                             mybir.ActivationFunctionType.Exp)

        # Broadcast col scales: Bb (128, 200) = ones_col @ inv_q
        ps_b = psum.tile([P, N], F32, name="ps_b", tag="ps")
        nc.tensor.matmul(ps_b, ones[0:1, :], inv["q"], start=True, stop=True)
        Bb = sb.tile([P, N], F32)
        nc.vector.tensor_copy(Bb, ps_b)

        # Diag row: d0[j] = sum_ch SkT[ch,j]*SqT[ch,j]; then scale
        ps_d = psum.tile([1, N], F32, name="ps_d", tag="ps")
        for cb in range(2):
            dk = sb.tile([P, N], F32, name="dk")
            nc.vector.tensor_mul(dk, ST[("k", cb)], ST[("q", cb)])
            nc.tensor.matmul(ps_d, ones[:, 0:1], dk, start=(cb == 0), stop=(cb == 1))
        drow = sb.tile([1, N], F32)
        nc.vector.tensor_mul(drow, ps_d, invk_tau)
        nc.vector.tensor_mul(drow, drow, inv["q"])

        # pad row: SkT[0, :] != 0
        padrow = sb.tile([1, N], F32)
        nc.vector.tensor_scalar(padrow, ST[("k", 0)][0:1, :], 0.0, None,
                                op0=mybir.AluOpType.not_equal)

        # Per row-block m: logits, lse, ce, masked sums
        nd_ps = psum_nd.tile([1, 2], F32, name="nd_ps")
        blocks = [(0, P), (P, N - P)]  # (start, rows)
        for mi, (i0, rows) in enumerate(blocks):
            ps_L = psum.tile([P, N], F32, name="ps_L", tag="ps")
            for cb in range(2):
                nc.tensor.matmul(ps_L[:rows, :], ST[("k", cb)][:, i0:i0 + rows],
                                 ST[("q", cb)], start=(cb == 0), stop=(cb == 1))
            # per-row scale a_i = invk_tau[i] as column
            acol_ps = psum.tile([P, 1], F32, name="acol_ps", tag="ps")
            nc.tensor.transpose(acol_ps[:rows, :], invk_tau[:, i0:i0 + rows], ident[0:1, 0:1])
            acol = sb.tile([P, 1], F32, name="acol")
            nc.vector.tensor_copy(acol[:rows], acol_ps[:rows])
            # logits = (raw * a_i) * b_j  in one fused DVE op
            lg = sb.tile([P, N], F32, name="lg")
            nc.vector.scalar_tensor_tensor(lg[:rows], ps_L[:rows, :], acol[:rows],
                                           Bb[:rows], op0=mybir.AluOpType.mult,
                                           op1=mybir.AluOpType.mult)
            # lse without max subtraction (|logits| <= ~14.3 is exp-safe)
            es = sb.tile([P, N], F32, name="es")
            ssum = sb.tile([P, 1], F32, name="ssum")
            nc.scalar.activation(es[:rows], lg[:rows],
                                 mybir.ActivationFunctionType.Exp,
                                 accum_out=ssum[:rows])
            lse = sb.tile([P, 1], F32, name="lse")
            nc.scalar.activation(lse[:rows], ssum[:rows],
                                 mybir.ActivationFunctionType.Ln)

            # diag + pad as columns (two K=1 transposes)
            d_ps = psum.tile([P, 1], F32, name="d_ps", tag="ps")
            nc.tensor.transpose(d_ps[:rows, :], drow[:, i0:i0 + rows], ident[0:1, 0:1])
            p_ps = psum.tile([P, 1], F32, name="p_ps", tag="ps")
            nc.tensor.transpose(p_ps[:rows, :], padrow[:, i0:i0 + rows], ident[0:1, 0:1])
            dcol = sb.tile([P, 1], F32, name="dcol")
            nc.vector.tensor_copy(dcol[:rows], d_ps[:rows])
            pcol = sb.tile([P, 1], F32, name="pcol")
            nc.vector.tensor_copy(pcol[:rows], p_ps[:rows])

            ce = sb.tile([P, 2], F32, name="ce")
            # ce[:,0] = (lse - d) * pad ; ce[:,1] = pad
            nc.vector.scalar_tensor_tensor(ce[:rows, 0:1], lse[:rows], dcol[:rows],
                                           pcol[:rows], op0=mybir.AluOpType.subtract,
                                           op1=mybir.AluOpType.mult)
            nc.vector.tensor_copy(ce[:rows, 1:2], pcol[:rows])
            nc.tensor.matmul(nd_ps, ones[:rows, 0:1], ce[:rows],
                             start=(mi == 0), stop=(mi == 1))

        den = sb.tile([1, 1], F32)
        nc.vector.tensor_scalar_max(den, nd_ps[:, 1:2], 1.0)
        rden = sb.tile([1, 1], F32)
        nc.vector.reciprocal(rden, den)
        res = sb.tile([1, 1], F32)
        nc.vector.tensor_mul(res, nd_ps[:, 0:1], rden)
        nc.sync.dma_start(out=out[:], in_=res)
    nc.compile()
    return nc


def _host_prep(features_q, features_k, pos_region_ranges):
    """Shard inputs (pure slicing / layout permutation / dtype packing)."""
    fq = np.ascontiguousarray(np.asarray(features_q, dtype=np.float32)).reshape(B, C, HW)
    fk = np.ascontiguousarray(np.asarray(features_k, dtype=np.float32)).reshape(B, C, HW)
    mask = np.asarray(pos_region_ranges).astype(bool).reshape(B, M, HW)
    mask_flat = mask.reshape(N, HW)

    in_maps = []
    for core in range(8):
        r, half = core // 2, core % 2
        lo = half * PX
        hi = min(lo + PX, HW)
        n = hi - lo

        def shard_feat(f):
            out = np.zeros((C, PX), np.float32)
            out[:, :n] = f[r, :, lo:hi]
            return out

        def shard_mask(rows):  # rows: (50, HW) bool
            t = np.zeros((Q, PX), NP_FP8)
            t[:, :n] = rows[:, lo:hi].astype(NP_FP8)
            # (50, T*128) -> (50, T, 128) -> (128, T, 50)
            return np.ascontiguousarray(t.reshape(Q, T, P).transpose(2, 1, 0))

        in_maps.append({
            "fq": shard_feat(fq),
            "fk": shard_feat(fk),
            "mat": shard_mask(mask_flat[r::4]),      # mA rows i = q*4+r
            "mbt": shard_mask(mask[r]),              # mB rows = mask[r, q]
        })
    return in_maps


def kernel(features_q, features_k, pos_region_ranges):
    if "p1" not in _cache:
        _cache["p1"] = _build_phase1()
        _cache["p2"] = _build_phase2()
    nc1, nc2 = _cache["p1"], _cache["p2"]

    in_maps = _host_prep(features_q, features_k, pos_region_ranges)
    r1 = run_bass_kernel_spmd(nc1, in_maps, core_ids=list(range(8)))

    pq = np.stack([r1.results[i]["outq"] for i in range(8)])  # (8, 50, 256)
    pk = np.stack([r1.results[i]["outk"] for i in range(8)])
    r2 = run_bass_kernel_spmd(nc2, [{"pq": pq, "pk": pk}], core_ids=[0])
    loss = r2.results[0]["loss"][0, 0]
    return np.float32(loss)



# revision 5
# speedup vs baseline: 7.5062x; 7.5062x over previous
"""Trainium2 Bass kernel for nn_ContrastiveLoss (segment_reduce).

Strategy (8 NeuronCores, SPMD, 2 launches):
  The loss only depends on the masked segment means, and a comb row
  (mA_i & mB_i) is usually empty or touches a tiny fraction of the image.
  Host-side *mask-only* analysis finds
    - the valid objects (nonzero comb rows; the rest enter the loss only as
      exp(0)=1 constants inside the logsumexp, folded into the Ln bias), and
    - the 128-px pixel tiles each valid comb row touches.
  Work items (batch, tile) are distributed round-robin over the 8 cores.

  Phase 1 (8 cores): each core gets its feature tiles pre-transposed to
    (pixel-partition, channel) fp8 plus the matching comb tiles, and
    accumulates psT[ch, v] += ft_tile^T-contracted-over-pixels via PE matmuls
    (output free size = Vp, the padded valid count). Output: (128, 4*Vp) bf16
    partials [nm(q,k) x cb(2 channel blocks)].
  Phase 2 (1 core): one DMA of all 8 cores' partials, reduce over the core
    axis, row norms, Vp x Vp logit block, ce = lse - diag, masked mean.

  fp8 features + bf16 partials keep rel err ~4e-3 (tolerance 2e-2); the /cnt
  of the reference cancels inside l2norm, and pad == (sk[:,0] != 0) is
  reproduced exactly on-chip.
"""

import math
import numpy as np
from contextlib import ExitStack

import concourse.bass as bass
import concourse.tile as tile
from concourse import bacc, mybir
from concourse.bass_utils import run_bass_kernel_spmd

# Problem constants (hardcoded per task spec)
B, M, C, H, W = 4, 50, 256, 100, 352
HW = H * W                  # 35200
N = B * M                   # 200
TAU = 0.07
P = 128                     # partitions / pixel tile
NT_FULL = HW // P           # 275 pixel tiles cover the image exactly

F32 = mybir.dt.float32
BF16 = mybir.dt.bfloat16
FP8 = mybir.dt.float8e4
NP_FP8 = mybir.dt.np(FP8)
NP_BF16 = mybir.dt.np(BF16)

# Force exp/ln to resolve to the combined "natural_log_exp_and_others" table
# set (index 6): empty the earlier sets we never want so first-match lands on
# sqrt_and_others (3) for sqrt/copy and natural_log_exp_and_others (6) for
# exp+ln. Indices are preserved so act_func_set_id stays aligned.
import concourse.bacc as _bacc_mod
import concourse.hw_specs as _hw_specs
_orig_get_tables = _hw_specs.get_activation_tables


def _patched_get_tables(module_arch):
    tables = dict(_orig_get_tables(module_arch))
    for i, k in enumerate(tables):
        if i in (0, 1, 2, 4, 5):
            tables[k] = set()
    return tables


_bacc_mod.get_activation_tables = _patched_get_tables

_cache = {}


def _build_phase1(T, Vp):
    """Per-core segment partial sums over T (batch,tile) work items.

    Input  fin  (128, 2*T*256 + T*Vp) fp8:
           [q tiles (T,256) | k tiles (T,256) | comb tiles (T,Vp)] per px row.
    Output pout (128, 4*Vp) bf16: columns [q_cb0 | q_cb1 | k_cb0 | k_cb1].
    """
    QOFF, KOFF, COFF = 0, T * 256, 2 * T * 256
    FS = 2 * T * 256 + T * Vp
    nc = bacc.Bacc(None, target_bir_lowering=False, debug=False)
    with tile.TileContext(nc) as tc, ExitStack() as ctx:
        dram = ctx.enter_context(tc.tile_pool(name="dram", bufs=1, space="DRAM"))
        fin = dram.tile([P, FS], FP8, kind="ExternalInput", name="fin",
                        uniquify=False)
        pout = dram.tile([P, 4 * Vp], BF16, kind="ExternalOutput", name="pout",
                         uniquify=False)

        sb = ctx.enter_context(tc.tile_pool(name="sb", bufs=1))
        psum = ctx.enter_context(tc.tile_pool(name="psum", bufs=1, space="PSUM"))

        fin_sb = sb.tile([P, FS], FP8, name="fin_sb")
        nc.sync.dma_start(out=fin_sb, in_=fin[:])

        # one PSUM tile (= one bank) per accumulation group: interleaved open
        # accumulation groups within a single bank corrupt results
        ps = {(nm, cb): psum.tile([P, Vp], F32, name=f"ps{nm}{cb}")
              for nm in range(2) for cb in range(2)}
        for nm, foff in ((0, QOFF), (1, KOFF)):
            for cb in range(2):
                for t in range(T):
                    lo = foff + t * 256 + cb * P
                    cmb = fin_sb[:, COFF + t * Vp: COFF + (t + 1) * Vp]
                    nc.tensor.matmul(ps[(nm, cb)], fin_sb[:, lo:lo + P],
                                     cmb, start=(t == 0), stop=(t == T - 1))

        o = sb.tile([P, 2, 2, Vp], BF16, name="o")
        for cb in range(2):
            nc.vector.tensor_copy(o[:, 0, cb], ps[(0, cb)])
            nc.scalar.copy(o[:, 1, cb], ps[(1, cb)])
        nc.sync.dma_start(out=pout[:], in_=o)
    nc.compile()
    return nc


def _build_phase2(Vp):
    """Combine 8 cores' partials into the scalar loss.

    Input pin (128, 2, 2, Vp, 8) bf16  [ch, nm(q,k), cb, v, core].
    Invalid columns beyond the true valid count contribute exp(0)=1 each;
    the remaining (N - Vp) zero columns are folded into the Ln bias.
    """
    nc = bacc.Bacc(None, target_bir_lowering=False, debug=False)
    blocks = [(i0, min(P, Vp - i0)) for i0 in range(0, Vp, P)]
    with tile.TileContext(nc) as tc, ExitStack() as ctx:
        dram = ctx.enter_context(tc.tile_pool(name="dram", bufs=1, space="DRAM"))
        pin = dram.tile([P, 2, 2, Vp, 8], BF16, kind="ExternalInput",
                        name="pin", uniquify=False)
        out = dram.tile([1, 1], F32, kind="ExternalOutput", name="loss",
                        uniquify=False)

        sb = ctx.enter_context(tc.tile_pool(name="sb", bufs=1))
        psum = ctx.enter_context(tc.tile_pool(name="psum", bufs=3, space="PSUM"))

        ones = sb.tile([P, 1], F32, name="ones")
        nc.gpsimd.memset(ones[:], 1.0)
        onesr = sb.tile([1, P], F32, name="onesr")
        nc.gpsimd.memset(onesr[:], 1.0)
        one1 = onesr[0:1, 0:1]
        # per-block diagonal selectors: idb[bi][p, j] = (p + i0 == j)
        idbs = []
        for i0, rows in blocks:
            idb = sb.tile([P, Vp], F32, name=f"idb{i0}")
            nc.gpsimd.memset(idb[:], 0.0)
            nc.gpsimd.affine_select(
                out=idb[:], in_=idb[:],
                compare_op=mybir.AluOpType.not_equal, fill=1.0, base=i0,
                pattern=[[-1, Vp]], channel_multiplier=1)
            idbs.append(idb)
        # const biases: tiny epsilon under the sqrt (keeps zero rows finite
        # without a clamp op) and the Ln fold-in of the N - Vp zero columns
        ceps = sb.tile([1, 1], F32, name="ceps")
        nc.gpsimd.memset(ceps[:], 1e-30)
        czc = sb.tile([P, 1], F32, name="czc")
        nc.gpsimd.memset(czc[:], float(N - Vp))
        # warm both activation table sets during the input DMA
        w1 = sb.tile([1, 2], F32, name="w1")
        nc.scalar.sqrt(w1[:, 0:1], one1)
        nc.scalar.activation(w1[:, 1:2], one1,
                             mybir.ActivationFunctionType.Exp)

        raw = sb.tile([P, 2, 2, Vp, 8], BF16, name="raw")
        nc.sync.dma_start(out=raw, in_=pin[:])

        # ST[ch, nm, cb, v] = sum over cores
        ST = sb.tile([P, 2, 2, Vp], F32, name="ST")
        nc.vector.tensor_reduce(ST, raw, axis=mybir.AxisListType.X,
                                op=mybir.AluOpType.add)

        # row norms^2: nsq[0, nm, v] = sum_ch sum_cb ST^2
        sq = sb.tile([P, 2, 2, Vp], F32, name="sq")
        nc.vector.tensor_mul(sq, ST, ST)
        ns_ps = psum.tile([1, 2, Vp], F32, name="ns_ps", tag="ps")
        for cb in range(2):
            nc.tensor.matmul(ns_ps, ones, sq[:, :, cb, :],
                             start=(cb == 0), stop=(cb == 1))
        nrm = sb.tile([1, 2, Vp], F32, name="nrm")
        nc.scalar.activation(nrm, ns_ps, mybir.ActivationFunctionType.Sqrt,
                             bias=ceps)
        inv = sb.tile([1, 2, Vp], F32, name="inv")
        nc.vector.reciprocal(inv, nrm)

        invk_tau = sb.tile([1, Vp], F32, name="invk_tau")
        nc.vector.tensor_scalar_mul(invk_tau, inv[:, 1, :], 1.0 / TAU)
        padrow = sb.tile([1, Vp], F32, name="padrow")
        nc.vector.tensor_scalar(padrow, ST[0:1, 1, 0, :], 0.0, None,
                                op0=mybir.AluOpType.not_equal)

        nd = psum.tile([1, 2, len(blocks)], F32, name="nd", tag="ps")
        for bi, (i0, rows) in enumerate(blocks):
            # per-row scale a_i = invk_tau[i0+i], pad as columns
            a_ps = psum.tile([P, 1], F32, name="a_ps", tag="ps")
            nc.tensor.transpose(a_ps[:rows], invk_tau[:, i0:i0 + rows], one1)
            p_ps = psum.tile([P, 1], F32, name="p_ps", tag="ps")
            nc.tensor.transpose(p_ps[:rows], padrow[:, i0:i0 + rows], one1)
            acol = sb.tile([P, 1], F32, name="acol")
            nc.vector.tensor_copy(acol[:rows], a_ps[:rows])
            pcol = sb.tile([P, 1], F32, name="pcol")
            nc.vector.tensor_copy(pcol[:rows], p_ps[:rows])
            # per-col scale broadcast Bb[i, j] = inv_q[j]
            bb_ps = psum.tile([P, Vp], F32, name="bb_ps", tag="ps")
            nc.tensor.matmul(bb_ps[:rows], onesr[0:1, 0:rows], inv[:, 0, :],
                             start=True, stop=True)
            Bb = sb.tile([P, Vp], F32, name="Bb")
            nc.vector.tensor_copy(Bb[:rows], bb_ps[:rows])
            # raw logits for rows [i0, i0+rows)
            ps_L = psum.tile([P, Vp], F32, name="ps_L", tag="ps")
            for cb in range(2):
                nc.tensor.matmul(ps_L[:rows], ST[:, 1, cb, i0:i0 + rows],
                                 ST[:, 0, cb, :], start=(cb == 0),
                                 stop=(cb == 1))
            lg = sb.tile([P, Vp], F32, name="lg")
            nc.vector.scalar_tensor_tensor(lg[:rows], ps_L[:rows],
                                           acol[:rows], Bb[:rows],
                                           op0=mybir.AluOpType.mult,
                                           op1=mybir.AluOpType.mult)
            # lse (no max subtraction: |logits| <= ~14.3 is exp-safe);
            # ln bias folds the N - Vp all-zero columns (each exp(0) = 1)
            es = sb.tile([P, Vp], F32, name="es")
            ssum = sb.tile([P, 1], F32, name="ssum")
            nc.scalar.activation(es[:rows], lg[:rows],
                                 mybir.ActivationFunctionType.Exp,
                                 accum_out=ssum[:rows])
            lse = sb.tile([P, 1], F32, name="lse")
            nc.scalar.activation(lse[:rows], ssum[:rows],
                                 mybir.ActivationFunctionType.Ln,
                                 bias=czc[:rows])
            # diag via masked row-sum
            dsel = sb.tile([P, Vp], F32, name="dsel")
            nc.vector.tensor_mul(dsel[:rows], lg[:rows], idbs[bi][0:rows, :])
            dcol = sb.tile([P, 1], F32, name="dcol")
            nc.vector.tensor_reduce(dcol[:rows], dsel[:rows],
                                    axis=mybir.AxisListType.X,
                                    op=mybir.AluOpType.add)
            ce = sb.tile([P, 1], F32, name="ce")
            nc.vector.scalar_tensor_tensor(ce[:rows], lse[:rows], dcol[:rows],
                                           pcol[:rows],
                                           op0=mybir.AluOpType.subtract,
                                           op1=mybir.AluOpType.mult)
            nc.tensor.matmul(nd[:, 0, bi:bi + 1], ones[0:rows], ce[:rows],
                             start=True, stop=True)
            nc.tensor.matmul(nd[:, 1, bi:bi + 1], ones[0:rows], pcol[:rows],
                             start=True, stop=True)

        num = sb.tile([1, 2], F32, name="num")
        if len(blocks) == 1:
            nc.vector.tensor_copy(num[:, 0:1], nd[:, 0, :])
            nc.vector.tensor_copy(num[:, 1:2], nd[:, 1, :])
        else:
            nc.vector.tensor_reduce(num, nd, axis=mybir.AxisListType.X,
                                    op=mybir.AluOpType.add)
        den = sb.tile([1, 1], F32, name="den")
        nc.vector.tensor_scalar_max(den, num[:, 1:2], 1.0)
        rden = sb.tile([1, 1], F32, name="rden")
        nc.vector.reciprocal(rden, den)
        res = sb.tile([1, 1], F32, name="res")
        nc.vector.tensor_mul(res, num[:, 0:1], rden)
        nc.sync.dma_start(out=out[:], in_=res)
    nc.compile()
    return nc


def _analyze(mask):
    """Mask-only analysis: valid objects and their (batch, tile) work items."""
    mask2 = mask.reshape(B, M, HW)
    mask_flat = mask2.reshape(N, HW)
    ii = np.arange(N)
    comb = mask_flat & mask2[ii % B, ii // B]      # (N, HW)
    vidx = np.nonzero(comb.any(axis=1))[0]
    V = len(vidx)
    if V == 0:
        return comb, vidx, 0, 0, []
    Vp = min(N, max(8, -(-V // 8) * 8))
    combT = comb[vidx].reshape(V, NT_FULL, P).any(axis=2)   # (V, NT_FULL)
    items = []
    for r in range(B):
        sel = (vidx % B) == r
        if sel.any():
            for t in np.nonzero(combT[sel].any(axis=0))[0]:
                items.append((r, int(t)))
    return comb, vidx, V, Vp, items


def _host_prep(features_q, features_k, pos_region_ranges):
    """Shard mask-selected feature tiles + comb tiles across 8 cores."""
    fq = np.asarray(features_q, dtype=np.float32).reshape(B, C, HW)
    fk = np.asarray(features_k, dtype=np.float32).reshape(B, C, HW)
    mask = np.asarray(pos_region_ranges).astype(bool)

    comb, vidx, V, Vp, items = _analyze(mask)
    if V == 0:
        return None
    T = max(1, -(-len(items) // 8))
    FS = 2 * T * 256 + T * Vp

    combV = comb[vidx]                              # (V, HW)
    rsel = [(vidx % B) == r for r in range(B)]
    in_maps = []
    for core in range(8):
        fused = np.zeros((P, FS), NP_FP8)
        fq_v = fused[:, :T * 256].reshape(P, T, 256)
        fk_v = fused[:, T * 256:2 * T * 256].reshape(P, T, 256)
        cb_v = fused[:, 2 * T * 256:].reshape(P, T, Vp)
        for j, (r, t) in enumerate(items[core::8]):
            sl = slice(t * P, (t + 1) * P)
            fq_v[:, j, :] = fq[r][:, sl].T.astype(NP_FP8)
            fk_v[:, j, :] = fk[r][:, sl].T.astype(NP_FP8)
            ct = np.where(rsel[r][:, None], combV[:, sl], False)  # (V, 128)
            cb_v[:, j, :V] = ct.T.astype(NP_FP8)
        in_maps.append({"fin": fused})
    return in_maps, vidx, V, Vp, T


def kernel(features_q, features_k, pos_region_ranges):
    prep = _host_prep(features_q, features_k, pos_region_ranges)
    if prep is None:
        return np.float32(0.0)
    in_maps, vidx, V, Vp, T = prep

    key = (T, Vp)
    if key not in _cache:
        _cache[key] = (_build_phase1(T, Vp), _build_phase2(Vp))
    nc1, nc2 = _cache[key]
    _cache["p1"], _cache["p2"] = nc1, nc2   # for test harness introspection

    r1 = run_bass_kernel_spmd(nc1, in_maps, core_ids=list(range(8)))
    # (128, 2, 2, Vp, 8): stack core partials on the innermost axis
    pin = np.stack([np.asarray(r1.results[s]["pout"]) for s in range(8)],
                   axis=-1).reshape(P, 2, 2, Vp, 8)
    r2 = run_bass_kernel_spmd(nc2, [{"pin": pin}], core_ids=[0])
    loss = r2.results[0]["loss"][0, 0]
    return np.float32(loss)


# revision 10
# speedup vs baseline: 8.1563x; 1.0866x over previous
"""Trainium2 Bass kernel for nn_ContrastiveLoss (segment_reduce).

Strategy (8 NeuronCores, SPMD, 2 launches):
  The loss only depends on the masked segment means, and a comb row
  (mA_i & mB_i) is usually empty or touches a tiny fraction of the image.
  Host-side *mask-only* analysis finds
    - the valid objects (nonzero comb rows; the rest enter the loss only as
      exp(0)=1 constants inside the logsumexp, folded into the Ln bias), and
    - the 128-px pixel tiles each valid comb row touches.
  Work items (batch, tile) are distributed round-robin over the 8 cores.

  Phase 1 (8 cores): each core gets its feature tiles pre-transposed to
    (pixel-partition, channel) fp8 plus the matching comb tiles, and
    accumulates psT[ch, v] += ft_tile^T-contracted-over-pixels via PE matmuls
    (output free size = Vp, the padded valid count). Output: (128, 4*Vp) bf16
    partials [nm(q,k) x cb(2 channel blocks)].
  Phase 2 (1 core): one DMA of all 8 cores' partials, reduce over the core
    axis, row norms, Vp x Vp logit block, ce = lse - diag, masked mean.

  fp8 features + bf16 partials keep rel err ~4e-3 (tolerance 2e-2); the /cnt
  of the reference cancels inside l2norm, and pad == (sk[:,0] != 0) is
  reproduced exactly on-chip.
"""

import math
import numpy as np
from contextlib import ExitStack

import concourse.bass as bass
import concourse.tile as tile
from concourse import bacc, mybir
from concourse.bass_utils import run_bass_kernel_spmd

# Problem constants (hardcoded per task spec)
B, M, C, H, W = 4, 50, 256, 100, 352
HW = H * W                  # 35200
N = B * M                   # 200
TAU = 0.07
P = 128                     # partitions / pixel tile
NT_FULL = HW // P           # 275 pixel tiles cover the image exactly

F32 = mybir.dt.float32
BF16 = mybir.dt.bfloat16
FP8 = mybir.dt.float8e4
NP_FP8 = mybir.dt.np(FP8)
NP_BF16 = mybir.dt.np(BF16)

# Force exp/ln to resolve to the combined "natural_log_exp_and_others" table
# set (index 6): empty the earlier sets we never want so first-match lands on
# sqrt_and_others (3) for sqrt/copy and natural_log_exp_and_others (6) for
# exp+ln. Indices are preserved so act_func_set_id stays aligned.
import concourse.bacc as _bacc_mod
import concourse.hw_specs as _hw_specs
_orig_get_tables = _hw_specs.get_activation_tables


def _patched_get_tables(module_arch):
    tables = dict(_orig_get_tables(module_arch))
    for i, k in enumerate(tables):
        if i in (0, 1, 2, 4, 5):
            tables[k] = set()
    return tables


_bacc_mod.get_activation_tables = _patched_get_tables

_cache = {}


def _build_phase1(T, Vp):
    """Per-core segment partial sums over T (batch,tile) work items.

    Input  fin  (128, 2*T*256 + T*Vp) fp8:
           [q tiles (T,256) | comb tiles (T,Vp) | k tiles (T,256)] per px row.
    Output pout (128, 4*Vp) bf16: columns [q_cb0 | q_cb1 | k_cb0 | k_cb1].
    The q+comb regions arrive in the first DMA so q matmuls overlap the k
    transfer. Accumulation groups run back-to-back (an interleaved open
    group within a PSUM bank corrupts results), so one PSUM tile suffices.
    """
    QOFF, COFF, KOFF = 0, T * 256, T * 256 + T * Vp
    FS = 2 * T * 256 + T * Vp
    nc = bacc.Bacc(None, target_bir_lowering=False, debug=False)
    with tile.TileContext(nc) as tc, ExitStack() as ctx:
        dram = ctx.enter_context(tc.tile_pool(name="dram", bufs=1, space="DRAM"))
        fin = dram.tile([P, FS], FP8, kind="ExternalInput", name="fin",
                        uniquify=False)
        pout = dram.tile([P, 4 * Vp], BF16, kind="ExternalOutput", name="pout",
                         uniquify=False)

        sb = ctx.enter_context(tc.tile_pool(name="sb", bufs=1))
        psum = ctx.enter_context(tc.tile_pool(name="psum", bufs=1, space="PSUM"))

        fin_a = sb.tile([P, KOFF], FP8, name="fin_a")
        fin_b = sb.tile([P, T * 256], FP8, name="fin_b")
        nc.sync.dma_start(out=fin_a, in_=fin[:, :KOFF])
        nc.sync.dma_start(out=fin_b, in_=fin[:, KOFF:])

        ps = psum.tile([P, 2, 2, Vp], F32, name="ps")
        for nm, src, foff in ((0, fin_a, QOFF), (1, fin_b, 0)):
            for cb in range(2):
                for t in range(T):
                    lo = foff + t * 256 + cb * P
                    cmb = fin_a[:, COFF + t * Vp: COFF + (t + 1) * Vp]
                    nc.tensor.matmul(ps[:, nm, cb, :], src[:, lo:lo + P],
                                     cmb, start=(t == 0), stop=(t == T - 1))

        o = sb.tile([P, 2, 2, Vp], BF16, name="o")
        nc.vector.tensor_copy(o, ps)
        nc.sync.dma_start(out=pout[:], in_=o)
    nc.compile()
    return nc


def _build_phase2(Vp):
    """Combine 8 cores' partials into the scalar loss.

    Input pin (128, 2, 2, Vp, 8) bf16  [ch, nm(q,k), cb, v, core].
    Invalid columns beyond the true valid count contribute exp(0)=1 each;
    the remaining (N - Vp) zero columns are folded into the Ln bias.
    """
    nc = bacc.Bacc(None, target_bir_lowering=False, debug=False)
    blocks = [(i0, min(P, Vp - i0)) for i0 in range(0, Vp, P)]
    with tile.TileContext(nc) as tc, ExitStack() as ctx:
        dram = ctx.enter_context(tc.tile_pool(name="dram", bufs=1, space="DRAM"))
        pin = dram.tile([P, 2, 2, Vp, 8], BF16, kind="ExternalInput",
                        name="pin", uniquify=False)
        out = dram.tile([1, 1], F32, kind="ExternalOutput", name="loss",
                        uniquify=False)

        sb = ctx.enter_context(tc.tile_pool(name="sb", bufs=1))
        psum = ctx.enter_context(tc.tile_pool(name="psum", bufs=3, space="PSUM"))

        ones = sb.tile([P, 1], F32, name="ones")
        nc.gpsimd.memset(ones[:], 1.0)
        onesr = sb.tile([1, P], F32, name="onesr")
        nc.gpsimd.memset(onesr[:], 1.0)
        one1 = onesr[0:1, 0:1]
        itaur = sb.tile([1, P], F32, name="itaur")
        nc.gpsimd.memset(itaur[:], 1.0 / TAU)
        # per-block diagonal selectors: idb[bi][p, j] = (p + i0 == j)
        idbs = []
        for i0, rows in blocks:
            idb = sb.tile([P, Vp], F32, name=f"idb{i0}")
            nc.gpsimd.memset(idb[:], 0.0)
            nc.gpsimd.affine_select(
                out=idb[:], in_=idb[:],
                compare_op=mybir.AluOpType.not_equal, fill=1.0, base=i0,
                pattern=[[-1, Vp]], channel_multiplier=1)
            idbs.append(idb)
        # const biases: tiny epsilon under the sqrt (keeps zero rows finite
        # without a clamp op) and the Ln fold-in of the N - Vp zero columns
        ceps = sb.tile([1, 1], F32, name="ceps")
        nc.gpsimd.memset(ceps[:], 1e-30)
        czc = sb.tile([P, 1], F32, name="czc")
        nc.gpsimd.memset(czc[:], float(N - Vp))
        # warm the exp/ln table set during the input DMA; every Act op in
        # this kernel (Ln, Exp) lives in that single set, so no reloads
        w1 = sb.tile([1, 2], F32, name="w1")
        nc.scalar.activation(w1[:, 1:2], one1,
                             mybir.ActivationFunctionType.Exp)

        raw = sb.tile([P, 2, 2, Vp, 8], BF16, name="raw")
        nc.sync.dma_start(out=raw, in_=pin[:])

        # ST[ch, nm, cb, v] = sum over cores
        ST = sb.tile([P, 2, 2, Vp], F32, name="ST")
        nc.vector.tensor_reduce(ST, raw, axis=mybir.AxisListType.X,
                                op=mybir.AluOpType.add)

        # raw logit matmuls only need ST: issue them first so the in-order
        # PE queue runs them during the norm chain
        ps_Ls = []
        for bi, (i0, rows) in enumerate(blocks):
            ps_L = psum.tile([P, Vp], F32, name=f"ps_L{bi}", tag="ps")
            for cb in range(2):
                nc.tensor.matmul(ps_L[:rows], ST[:, 1, cb, i0:i0 + rows],
                                 ST[:, 0, cb, :], start=(cb == 0),
                                 stop=(cb == 1))
            ps_Ls.append(ps_L)

        # row norms^2: nsq[0, nm, v] = sum_ch sum_cb ST^2, then
        # 1/norm = exp(-0.5 * ln(nsq + eps)) — stays in the exp/ln table set
        sq = sb.tile([P, 2, 2, Vp], F32, name="sq")
        nc.vector.tensor_mul(sq, ST, ST)
        ns_ps = psum.tile([1, 2, Vp], F32, name="ns_ps", tag="ps")
        for cb in range(2):
            nc.tensor.matmul(ns_ps, ones, sq[:, :, cb, :],
                             start=(cb == 0), stop=(cb == 1))
        lnn = sb.tile([1, 2, Vp], F32, name="lnn")
        nc.scalar.activation(lnn, ns_ps, mybir.ActivationFunctionType.Ln,
                             bias=ceps)
        inv = sb.tile([1, 2, Vp], F32, name="inv")
        nc.scalar.activation(inv, lnn, mybir.ActivationFunctionType.Exp,
                             scale=-0.5)

        padrow = sb.tile([1, Vp], F32, name="padrow")
        nc.vector.tensor_scalar(padrow, ST[0:1, 1, 0, :], 0.0, None,
                                op0=mybir.AluOpType.not_equal)

        nd = psum.tile([1, 2, len(blocks)], F32, name="nd", tag="ps")
        for bi, (i0, rows) in enumerate(blocks):
            # per-row scale a_i = inv_k[i0+i], pad as columns
            a_ps = psum.tile([P, 1], F32, name="a_ps", tag="ps")
            nc.tensor.transpose(a_ps[:rows], inv[:, 1, i0:i0 + rows], one1)
            p_ps = psum.tile([P, 1], F32, name="p_ps", tag="ps")
            nc.tensor.transpose(p_ps[:rows], padrow[:, i0:i0 + rows], one1)
            acol = sb.tile([P, 1], F32, name="acol")
            nc.vector.tensor_copy(acol[:rows], a_ps[:rows])
            pcol = sb.tile([P, 1], F32, name="pcol")
            nc.vector.tensor_copy(pcol[:rows], p_ps[:rows])
            # per-col scale broadcast Bb[i, j] = inv_q[j] / tau
            bb_ps = psum.tile([P, Vp], F32, name="bb_ps", tag="ps")
            nc.tensor.matmul(bb_ps[:rows], itaur[0:1, 0:rows], inv[:, 0, :],
                             start=True, stop=True)
            Bb = sb.tile([P, Vp], F32, name="Bb")
            nc.vector.tensor_copy(Bb[:rows], bb_ps[:rows])
            ps_L = ps_Ls[bi]
            lg = sb.tile([P, Vp], F32, name="lg")
            nc.vector.scalar_tensor_tensor(lg[:rows], ps_L[:rows],
                                           acol[:rows], Bb[:rows],
                                           op0=mybir.AluOpType.mult,
                                           op1=mybir.AluOpType.mult)
            # lse (no max subtraction: |logits| <= ~14.3 is exp-safe);
            # ln bias folds the N - Vp all-zero columns (each exp(0) = 1)
            es = sb.tile([P, Vp], F32, name="es")
            ssum = sb.tile([P, 1], F32, name="ssum")
            nc.scalar.activation(es[:rows], lg[:rows],
                                 mybir.ActivationFunctionType.Exp,
                                 accum_out=ssum[:rows])
            lse = sb.tile([P, 1], F32, name="lse")
            nc.scalar.activation(lse[:rows], ssum[:rows],
                                 mybir.ActivationFunctionType.Ln,
                                 bias=czc[:rows])
            # diag via masked row-sum
            dsel = sb.tile([P, Vp], F32, name="dsel")
            nc.vector.tensor_mul(dsel[:rows], lg[:rows], idbs[bi][0:rows, :])
            dcol = sb.tile([P, 1], F32, name="dcol")
            nc.vector.tensor_reduce(dcol[:rows], dsel[:rows],
                                    axis=mybir.AxisListType.X,
                                    op=mybir.AluOpType.add)
            ce = sb.tile([P, 1], F32, name="ce")
            nc.vector.scalar_tensor_tensor(ce[:rows], lse[:rows], dcol[:rows],
                                           pcol[:rows],
                                           op0=mybir.AluOpType.subtract,
                                           op1=mybir.AluOpType.mult)
            nc.tensor.matmul(nd[:, 0, bi:bi + 1], ones[0:rows], ce[:rows],
                             start=True, stop=True)
            nc.tensor.matmul(nd[:, 1, bi:bi + 1], ones[0:rows], pcol[:rows],
                             start=True, stop=True)

        num = sb.tile([1, 2], F32, name="num")
        if len(blocks) == 1:
            nc.vector.tensor_copy(num[:, 0:1], nd[:, 0, :])
            nc.vector.tensor_copy(num[:, 1:2], nd[:, 1, :])
        else:
            nc.vector.tensor_reduce(num, nd, axis=mybir.AxisListType.X,
                                    op=mybir.AluOpType.add)
        den = sb.tile([1, 1], F32, name="den")
        nc.vector.tensor_scalar_max(den, num[:, 1:2], 1.0)
        rden = sb.tile([1, 1], F32, name="rden")
        nc.vector.reciprocal(rden, den)
        res = sb.tile([1, 1], F32, name="res")
        nc.vector.tensor_mul(res, num[:, 0:1], rden)
        nc.sync.dma_start(out=out[:], in_=res)
    nc.compile()
    return nc


def _analyze(mask):
    """Mask-only analysis: valid objects and their (batch, tile) work items."""
    mask2 = mask.reshape(B, M, HW)
    mask_flat = mask2.reshape(N, HW)
    ii = np.arange(N)
    comb = mask_flat & mask2[ii % B, ii // B]      # (N, HW)
    vidx = np.nonzero(comb.any(axis=1))[0]
    V = len(vidx)
    if V == 0:
        return comb, vidx, 0, 0, []
    Vp = min(N, max(8, -(-V // 8) * 8))
    combT = comb[vidx].reshape(V, NT_FULL, P).any(axis=2)   # (V, NT_FULL)
    items = []
    for r in range(B):
        sel = (vidx % B) == r
        if sel.any():
            for t in np.nonzero(combT[sel].any(axis=0))[0]:
                items.append((r, int(t)))
    return comb, vidx, V, Vp, items


def _host_prep(features_q, features_k, pos_region_ranges):
    """Shard mask-selected feature tiles + comb tiles across 8 cores."""
    fq = np.asarray(features_q, dtype=np.float32).reshape(B, C, HW)
    fk = np.asarray(features_k, dtype=np.float32).reshape(B, C, HW)
    mask = np.asarray(pos_region_ranges).astype(bool)

    comb, vidx, V, Vp, items = _analyze(mask)
    if V == 0:
        return None
    T = max(1, -(-len(items) // 8))
    FS = 2 * T * 256 + T * Vp

    combV = comb[vidx]                              # (V, HW)
    rsel = [(vidx % B) == r for r in range(B)]
    in_maps = []
    for core in range(8):
        fused = np.zeros((P, FS), NP_FP8)
        fq_v = fused[:, :T * 256].reshape(P, T, 256)
        cb_v = fused[:, T * 256:T * 256 + T * Vp].reshape(P, T, Vp)
        fk_v = fused[:, T * 256 + T * Vp:].reshape(P, T, 256)
        for j, (r, t) in enumerate(items[core::8]):
            sl = slice(t * P, (t + 1) * P)
            fq_v[:, j, :] = fq[r][:, sl].T.astype(NP_FP8)
            fk_v[:, j, :] = fk[r][:, sl].T.astype(NP_FP8)
            ct = np.where(rsel[r][:, None], combV[:, sl], False)  # (V, 128)
            cb_v[:, j, :V] = ct.T.astype(NP_FP8)
        in_maps.append({"fin": fused})
    return in_maps, vidx, V, Vp, T


def kernel(features_q, features_k, pos_region_ranges):
    prep = _host_prep(features_q, features_k, pos_region_ranges)
    if prep is None:
        return np.float32(0.0)
    in_maps, vidx, V, Vp, T = prep

    key = (T, Vp)
    if key not in _cache:
        _cache[key] = (_build_phase1(T, Vp), _build_phase2(Vp))
    nc1, nc2 = _cache[key]
    _cache["p1"], _cache["p2"] = nc1, nc2   # for test harness introspection

    r1 = run_bass_kernel_spmd(nc1, in_maps, core_ids=list(range(8)))
    # (128, 2, 2, Vp, 8): stack core partials on the innermost axis
    pin = np.stack([np.asarray(r1.results[s]["pout"]) for s in range(8)],
                   axis=-1).reshape(P, 2, 2, Vp, 8)
    r2 = run_bass_kernel_spmd(nc2, [{"pin": pin}], core_ids=[0])
    loss = r2.results[0]["loss"][0, 0]
    return np.float32(loss)


# revision 13
# speedup vs baseline: 12.2916x; 1.5070x over previous
"""Trainium2 Bass kernel for nn_ContrastiveLoss (segment_reduce).

The loss depends only on the masked segment means, and a comb row
(mA_i & mB_i) is usually empty or touches a tiny fraction of the image.
Host-side *mask-only* analysis finds
  - the valid objects (nonzero comb rows; the rest enter the loss only as
    exp(0)=1 constants inside the logsumexp, folded into the Ln bias), and
  - the 16-px pixel blocks each valid comb row touches; 8 blocks pack into
    one 128-partition contraction tile.

Fast path (sparse masks, the expected regime): ONE launch on one core —
the few selected feature/comb tiles stream in fp8, PE accumulates the
segment sums psT[ch, v] (output free size = Vp), and the loss chain
(norms via exp(-0.5*ln(nsq)), Vp x Vp logits, lse - diag, masked mean)
runs on the same core. No cross-core combine, no second-launch DMA
latency, full f32 partials.

Fallback (dense masks, > 128 tiles): two launches — 8 cores compute
bf16 partial sums data-parallel over tiles, core 0 combines.

fp8 features keep rel err ~4e-3 (tolerance 2e-2); the /cnt of the
reference cancels inside l2norm; pad == (sk[:,0] != 0) is reproduced
exactly on-chip. All Act ops (Ln, Exp) live in one table set, warmed
during the input DMA. PSUM accumulation groups never interleave within
a bank (that corrupts results).
"""

import numpy as np
from contextlib import ExitStack

import concourse.bass as bass
import concourse.tile as tile
from concourse import bacc, mybir
from concourse.bass_utils import run_bass_kernel_spmd

# Problem constants (hardcoded per task spec)
B, M, C, H, W = 4, 50, 256, 100, 352
HW = H * W                  # 35200
N = B * M                   # 200
TAU = 0.07
P = 128                     # partitions / contraction tile
BS = 16                     # selection block: 16 consecutive pixels
BPT = P // BS               # blocks packed per tile
NB_FULL = HW // BS

F32 = mybir.dt.float32
BF16 = mybir.dt.bfloat16
FP8 = mybir.dt.float8e4
NP_FP8 = mybir.dt.np(FP8)

# Force exp/ln to resolve to the combined "natural_log_exp_and_others" table
# set (index 6): empty the earlier sets we never want so first-match lands on
# sqrt_and_others (3) for sqrt/copy and natural_log_exp_and_others (6) for
# exp+ln. Indices are preserved so act_func_set_id stays aligned.
import concourse.bacc as _bacc_mod
import concourse.hw_specs as _hw_specs
_orig_get_tables = _hw_specs.get_activation_tables


def _patched_get_tables(module_arch):
    tables = dict(_orig_get_tables(module_arch))
    for i, k in enumerate(tables):
        if i in (0, 1, 2, 4, 5):
            tables[k] = set()
    return tables


_bacc_mod.get_activation_tables = _patched_get_tables

_cache = {}


def _emit_consts(nc, sb, Vp, blocks):
    """Constant tiles + activation warm-up (all overlap the input DMA)."""
    ones = sb.tile([P, 1], F32, name="ones")
    nc.gpsimd.memset(ones[:], 1.0)
    onesr = sb.tile([1, P], F32, name="onesr")
    nc.gpsimd.memset(onesr[:], 1.0)
    itaur = sb.tile([1, P], F32, name="itaur")
    nc.gpsimd.memset(itaur[:], 1.0 / TAU)
    idbs = []
    for i0, rows in blocks:
        idb = sb.tile([P, Vp], F32, name=f"idb{i0}")
        nc.gpsimd.memset(idb[:], 0.0)
        nc.gpsimd.affine_select(
            out=idb[:], in_=idb[:],
            compare_op=mybir.AluOpType.not_equal, fill=1.0, base=i0,
            pattern=[[-1, Vp]], channel_multiplier=1)
        idbs.append(idb)
    ceps = sb.tile([1, 1], F32, name="ceps")
    nc.gpsimd.memset(ceps[:], 1e-30)
    czc = sb.tile([P, 1], F32, name="czc")
    nc.gpsimd.memset(czc[:], float(N - Vp))
    # warm the exp/ln table set; every Act op here lives in that single set
    w1 = sb.tile([1, 1], F32, name="w1")
    nc.scalar.activation(w1, onesr[0:1, 0:1],
                         mybir.ActivationFunctionType.Exp)
    return dict(ones=ones, onesr=onesr, itaur=itaur, idbs=idbs,
                ceps=ceps, czc=czc)


def _emit_loss(nc, sb, psum, ST, Vp, cst, out):
    """Loss from ST (128ch, nm, cb, Vp) f32 segment sums; DMAs to `out`."""
    blocks = [(i0, min(P, Vp - i0)) for i0 in range(0, Vp, P)]
    one1 = cst["onesr"][0:1, 0:1]

    # raw logit matmuls only need ST: issue them first so the in-order
    # PE queue runs them during the norm chain
    ps_Ls = []
    for bi, (i0, rows) in enumerate(blocks):
        ps_L = psum.tile([P, Vp], F32, name=f"ps_L{bi}", tag="ps")
        for cb in range(2):
            nc.tensor.matmul(ps_L[:rows], ST[:, 1, cb, i0:i0 + rows],
                             ST[:, 0, cb, :], start=(cb == 0),
                             stop=(cb == 1))
        ps_Ls.append(ps_L)

    # row norms^2, then 1/norm = exp(-0.5 * ln(nsq + eps))
    sq = sb.tile([P, 2, 2, Vp], F32, name="sq")
    nc.vector.tensor_mul(sq, ST, ST)
    ns_ps = psum.tile([1, 2, Vp], F32, name="ns_ps", tag="ps")
    for cb in range(2):
        nc.tensor.matmul(ns_ps, cst["ones"], sq[:, :, cb, :],
                         start=(cb == 0), stop=(cb == 1))
    lnn = sb.tile([1, 2, Vp], F32, name="lnn")
    nc.scalar.activation(lnn, ns_ps, mybir.ActivationFunctionType.Ln,
                         bias=cst["ceps"])
    inv = sb.tile([1, 2, Vp], F32, name="inv")
    nc.scalar.activation(inv, lnn, mybir.ActivationFunctionType.Exp,
                         scale=-0.5)

    padrow = sb.tile([1, Vp], F32, name="padrow")
    nc.vector.tensor_scalar(padrow, ST[0:1, 1, 0, :], 0.0, None,
                            op0=mybir.AluOpType.not_equal)

    nd = psum.tile([1, 2, len(blocks)], F32, name="nd", tag="ps")
    for bi, (i0, rows) in enumerate(blocks):
        # a_i = inv_k[i0+i] / tau (the 1/tau rides the psum->sbuf copy;
        # PE transpose is pure data movement and ignores identity values)
        a_ps = psum.tile([P, 1], F32, name="a_ps", tag="ps")
        nc.tensor.transpose(a_ps[:rows], inv[:, 1, i0:i0 + rows], one1)
        p_ps = psum.tile([P, 1], F32, name="p_ps", tag="ps")
        nc.tensor.transpose(p_ps[:rows], padrow[:, i0:i0 + rows], one1)
        acol = sb.tile([P, 1], F32, name="acol")
        nc.vector.tensor_scalar_mul(acol[:rows], a_ps[:rows], 1.0 / TAU)
        pcol = sb.tile([P, 1], F32, name="pcol")
        nc.vector.tensor_copy(pcol[:rows], p_ps[:rows])
        # per-col scale Bb[i, j] = inv_q[j] on the (otherwise idle) Pool
        Bb = sb.tile([P, Vp], F32, name="Bb")
        nc.gpsimd.partition_broadcast(Bb[:rows], inv[:, 0, :])
        lg = sb.tile([P, Vp], F32, name="lg")
        nc.vector.scalar_tensor_tensor(lg[:rows], ps_Ls[bi][:rows],
                                       acol[:rows], Bb[:rows],
                                       op0=mybir.AluOpType.mult,
                                       op1=mybir.AluOpType.mult)
        # lse without max subtraction (|logits| <= ~14.3 is exp-safe);
        # Ln bias folds the N - Vp all-zero columns (each exp(0) = 1)
        es = sb.tile([P, Vp], F32, name="es")
        ssum = sb.tile([P, 1], F32, name="ssum")
        nc.scalar.activation(es[:rows], lg[:rows],
                             mybir.ActivationFunctionType.Exp,
                             accum_out=ssum[:rows])
        lse = sb.tile([P, 1], F32, name="lse")
        nc.scalar.activation(lse[:rows], ssum[:rows],
                             mybir.ActivationFunctionType.Ln,
                             bias=cst["czc"][:rows])
        # diag via masked row-sum
        dsel = sb.tile([P, Vp], F32, name="dsel")
        nc.vector.tensor_mul(dsel[:rows], lg[:rows], cst["idbs"][bi][0:rows, :])
        dcol = sb.tile([P, 1], F32, name="dcol")
        nc.vector.tensor_reduce(dcol[:rows], dsel[:rows],
                                axis=mybir.AxisListType.X,
                                op=mybir.AluOpType.add)
        ce = sb.tile([P, 1], F32, name="ce")
        nc.vector.scalar_tensor_tensor(ce[:rows], lse[:rows], dcol[:rows],
                                       pcol[:rows],
                                       op0=mybir.AluOpType.subtract,
                                       op1=mybir.AluOpType.mult)
        nc.tensor.matmul(nd[:, 0, bi:bi + 1], cst["ones"][0:rows], ce[:rows],
                         start=True, stop=True)
        nc.tensor.matmul(nd[:, 1, bi:bi + 1], cst["ones"][0:rows],
                         pcol[:rows], start=True, stop=True)

    num = sb.tile([1, 2], F32, name="num")
    if len(blocks) == 1:
        nc.vector.tensor_copy(num[:, 0:1], nd[:, 0, :])
        nc.vector.tensor_copy(num[:, 1:2], nd[:, 1, :])
    else:
        nc.vector.tensor_reduce(num, nd, axis=mybir.AxisListType.X,
                                op=mybir.AluOpType.add)
    den = sb.tile([1, 1], F32, name="den")
    nc.vector.tensor_scalar_max(den, num[:, 1:2], 1.0)
    rden = sb.tile([1, 1], F32, name="rden")
    nc.vector.reciprocal(rden, den)
    res = sb.tile([1, 1], F32, name="res")
    nc.vector.tensor_mul(res, num[:, 0:1], rden)
    nc.sync.dma_start(out=out[:], in_=res)


def _emit_segsum_matmuls(nc, fin_a, fin_b, psum, T, Vp):
    """Accumulate psT[ch, nm, cb, v] over T tiles. fin_a = [q | comb],
    fin_b = [k]. Groups run back-to-back (interleaved open groups within
    a PSUM bank corrupt results)."""
    COFF = T * 256
    ps = psum.tile([P, 2, 2, Vp], F32, name="ps")
    for nm, src in ((0, fin_a), (1, fin_b)):
        for cb in range(2):
            for t in range(T):
                lo = t * 256 + cb * P
                cmb = fin_a[:, COFF + t * Vp: COFF + (t + 1) * Vp]
                nc.tensor.matmul(ps[:, nm, cb, :], src[:, lo:lo + P],
                                 cmb, start=(t == 0), stop=(t == T - 1))
    return ps


def _build_single(T, Vp):
    """One launch, one core: segment sums + loss."""
    FS = 2 * T * 256 + T * Vp
    KOFF = T * 256 + T * Vp
    nc = bacc.Bacc(None, target_bir_lowering=False, debug=False)
    blocks = [(i0, min(P, Vp - i0)) for i0 in range(0, Vp, P)]
    with tile.TileContext(nc) as tc, ExitStack() as ctx:
        dram = ctx.enter_context(tc.tile_pool(name="dram", bufs=1, space="DRAM"))
        fin = dram.tile([P, FS], FP8, kind="ExternalInput", name="fin",
                        uniquify=False)
        out = dram.tile([1, 1], F32, kind="ExternalOutput", name="loss",
                        uniquify=False)
        sb = ctx.enter_context(tc.tile_pool(name="sb", bufs=1))
        psum = ctx.enter_context(tc.tile_pool(name="psum", bufs=3, space="PSUM"))

        cst = _emit_consts(nc, sb, Vp, blocks)

        fin_a = sb.tile([P, KOFF], FP8, name="fin_a")
        fin_b = sb.tile([P, T * 256], FP8, name="fin_b")
        nc.sync.dma_start(out=fin_a, in_=fin[:, :KOFF])
        nc.sync.dma_start(out=fin_b, in_=fin[:, KOFF:])

        ps = _emit_segsum_matmuls(nc, fin_a, fin_b, psum, T, Vp)
        ST = sb.tile([P, 2, 2, Vp], F32, name="ST")
        nc.vector.tensor_copy(ST, ps)
        _emit_loss(nc, sb, psum, ST, Vp, cst, out)
    nc.compile()
    return nc


def _build_phase1(T, Vp):
    """Fallback launch 1 (8 cores): bf16 partial segment sums."""
    FS = 2 * T * 256 + T * Vp
    KOFF = T * 256 + T * Vp
    nc = bacc.Bacc(None, target_bir_lowering=False, debug=False)
    with tile.TileContext(nc) as tc, ExitStack() as ctx:
        dram = ctx.enter_context(tc.tile_pool(name="dram", bufs=1, space="DRAM"))
        fin = dram.tile([P, FS], FP8, kind="ExternalInput", name="fin",
                        uniquify=False)
        pout = dram.tile([P, 4 * Vp], BF16, kind="ExternalOutput", name="pout",
                         uniquify=False)
        sb = ctx.enter_context(tc.tile_pool(name="sb", bufs=1))
        psum = ctx.enter_context(tc.tile_pool(name="psum", bufs=1, space="PSUM"))

        fin_a = sb.tile([P, KOFF], FP8, name="fin_a")
        fin_b = sb.tile([P, T * 256], FP8, name="fin_b")
        nc.sync.dma_start(out=fin_a, in_=fin[:, :KOFF])
        nc.sync.dma_start(out=fin_b, in_=fin[:, KOFF:])

        ps = _emit_segsum_matmuls(nc, fin_a, fin_b, psum, T, Vp)
        o = sb.tile([P, 2, 2, Vp], BF16, name="o")
        nc.vector.tensor_copy(o, ps)
        nc.sync.dma_start(out=pout[:], in_=o)
    nc.compile()
    return nc


def _build_phase2(Vp):
    """Fallback launch 2 (1 core): combine 8 cores' partials into the loss."""
    nc = bacc.Bacc(None, target_bir_lowering=False, debug=False)
    blocks = [(i0, min(P, Vp - i0)) for i0 in range(0, Vp, P)]
    with tile.TileContext(nc) as tc, ExitStack() as ctx:
        dram = ctx.enter_context(tc.tile_pool(name="dram", bufs=1, space="DRAM"))
        pin = dram.tile([P, 2, 2, Vp, 8], BF16, kind="ExternalInput",
                        name="pin", uniquify=False)
        out = dram.tile([1, 1], F32, kind="ExternalOutput", name="loss",
                        uniquify=False)
        sb = ctx.enter_context(tc.tile_pool(name="sb", bufs=1))
        psum = ctx.enter_context(tc.tile_pool(name="psum", bufs=3, space="PSUM"))

        cst = _emit_consts(nc, sb, Vp, blocks)
        raw = sb.tile([P, 2, 2, Vp, 8], BF16, name="raw")
        nc.sync.dma_start(out=raw, in_=pin[:])
        ST = sb.tile([P, 2, 2, Vp], F32, name="ST")
        nc.vector.tensor_reduce(ST, raw, axis=mybir.AxisListType.X,
                                op=mybir.AluOpType.add)
        _emit_loss(nc, sb, psum, ST, Vp, cst, out)
    nc.compile()
    return nc


def _analyze(mask):
    """Mask-only analysis: valid objects and their (batch, block) work items."""
    mask2 = mask.reshape(B, M, HW)
    mask_flat = mask2.reshape(N, HW)
    ii = np.arange(N)
    comb = mask_flat & mask2[ii % B, ii // B]      # (N, HW)
    vidx = np.nonzero(comb.any(axis=1))[0]
    V = len(vidx)
    if V == 0:
        return comb, vidx, 0, 0, []
    Vp = min(N, max(8, -(-V // 8) * 8))
    combT = comb[vidx].reshape(V, NB_FULL, BS).any(axis=2)  # (V, NB_FULL)
    items = []
    for r in range(B):
        sel = (vidx % B) == r
        if sel.any():
            for t in np.nonzero(combT[sel].any(axis=0))[0]:
                items.append((r, int(t)))
    return comb, vidx, V, Vp, items


def _host_prep(features_q, features_k, pos_region_ranges):
    """Pack mask-selected 16-px feature/comb blocks into contraction tiles."""
    fq = np.asarray(features_q, dtype=np.float32).reshape(B, C, HW)
    fk = np.asarray(features_k, dtype=np.float32).reshape(B, C, HW)
    mask = np.asarray(pos_region_ranges).astype(bool)

    comb, vidx, V, Vp, items = _analyze(mask)
    if V == 0:
        return None
    total_tiles = max(1, -(-len(items) // BPT))
    single = total_tiles <= 128
    ncores = 1 if single else 8
    tiles = [items[j * BPT:(j + 1) * BPT] for j in range(total_tiles)]
    percore = [tiles[c::ncores] for c in range(ncores)]
    T = max(len(pc) for pc in percore)
    FS = 2 * T * 256 + T * Vp

    combV = comb[vidx]                              # (V, HW)
    rsel = [(vidx % B) == r for r in range(B)]
    in_maps = []
    for c in range(ncores):
        fused = np.zeros((P, FS), NP_FP8)
        fq_v = fused[:, :T * 256].reshape(P, T, 256)
        cb_v = fused[:, T * 256:T * 256 + T * Vp].reshape(P, T, Vp)
        fk_v = fused[:, T * 256 + T * Vp:].reshape(P, T, 256)
        for j, blks in enumerate(percore[c]):
            for bi, (r, blk) in enumerate(blks):
                rows = slice(bi * BS, (bi + 1) * BS)
                sl = slice(blk * BS, (blk + 1) * BS)
                fq_v[rows, j, :] = fq[r][:, sl].T.astype(NP_FP8)
                fk_v[rows, j, :] = fk[r][:, sl].T.astype(NP_FP8)
                ct = np.where(rsel[r][:, None], combV[:, sl], False)
                cb_v[rows, j, :V] = ct.T.astype(NP_FP8)
        in_maps.append({"fin": fused})
    return in_maps, V, Vp, T, single


def kernel(features_q, features_k, pos_region_ranges):
    prep = _host_prep(features_q, features_k, pos_region_ranges)
    if prep is None:
        return np.float32(0.0)
    in_maps, V, Vp, T, single = prep

    if single:
        key = ("s", T, Vp)
        if key not in _cache:
            _cache[key] = _build_single(T, Vp)
        nc = _cache[key]
        _cache["single"] = nc
        _cache.pop("p1", None); _cache.pop("p2", None)
        r = run_bass_kernel_spmd(nc, in_maps, core_ids=[0])
        return np.float32(r.results[0]["loss"][0, 0])

    key = ("d", T, Vp)
    if key not in _cache:
        _cache[key] = (_build_phase1(T, Vp), _build_phase2(Vp))
    nc1, nc2 = _cache[key]
    _cache["p1"], _cache["p2"] = nc1, nc2
    _cache.pop("single", None)
    r1 = run_bass_kernel_spmd(nc1, in_maps, core_ids=list(range(8)))
    pin = np.stack([np.asarray(r1.results[s]["pout"]) for s in range(8)],
                   axis=-1).reshape(P, 2, 2, Vp, 8)
    r2 = run_bass_kernel_spmd(nc2, [{"pin": pin}], core_ids=[0])
    return np.float32(r2.results[0]["loss"][0, 0])


# revision 14
# speedup vs baseline: 13.6899x; 1.1138x over previous
"""Trainium2 Bass kernel for nn_ContrastiveLoss (segment_reduce).

The loss depends only on the masked segment means, and a comb row
(mA_i & mB_i) is usually empty or touches a tiny fraction of the image.
Host-side *mask-only* analysis finds
  - the valid objects (nonzero comb rows; the rest enter the loss only as
    exp(0)=1 constants inside the logsumexp, folded into the Ln bias), and
  - the 16-px pixel blocks each valid comb row touches; 8 blocks pack into
    one 128-partition contraction tile.

Fast path (sparse masks, the expected regime): ONE launch on one core —
the few selected feature/comb tiles stream in fp8, PE accumulates the
segment sums psT[ch, v] (output free size = Vp), and the loss chain
(norms via exp(-0.5*ln(nsq)), Vp x Vp logits, lse - diag, masked mean)
runs on the same core. No cross-core combine, no second-launch DMA
latency, full f32 partials.

Fallback (dense masks, > 128 tiles): two launches — 8 cores compute
bf16 partial sums data-parallel over tiles, core 0 combines.

fp8 features keep rel err ~4e-3 (tolerance 2e-2); the /cnt of the
reference cancels inside l2norm; pad == (sk[:,0] != 0) is reproduced
exactly on-chip. All Act ops (Ln, Exp) live in one table set, warmed
during the input DMA. PSUM accumulation groups never interleave within
a bank (that corrupts results).
"""

import numpy as np
from contextlib import ExitStack

import concourse.bass as bass
import concourse.tile as tile
from concourse import bacc, mybir
from concourse.bass_utils import run_bass_kernel_spmd

# Problem constants (hardcoded per task spec)
B, M, C, H, W = 4, 50, 256, 100, 352
HW = H * W                  # 35200
N = B * M                   # 200
TAU = 0.07
P = 128                     # partitions / contraction tile
BS = 8                      # selection block: 8 consecutive pixels
BPT = P // BS               # blocks packed per tile
NB_FULL = HW // BS

F32 = mybir.dt.float32
BF16 = mybir.dt.bfloat16
FP8 = mybir.dt.float8e4
NP_FP8 = mybir.dt.np(FP8)

# Force exp/ln to resolve to the combined "natural_log_exp_and_others" table
# set (index 6): empty the earlier sets we never want so first-match lands on
# sqrt_and_others (3) for sqrt/copy and natural_log_exp_and_others (6) for
# exp+ln. Indices are preserved so act_func_set_id stays aligned.
import concourse.bacc as _bacc_mod
import concourse.hw_specs as _hw_specs
_orig_get_tables = _hw_specs.get_activation_tables


def _patched_get_tables(module_arch):
    tables = dict(_orig_get_tables(module_arch))
    for i, k in enumerate(tables):
        if i in (0, 1, 2, 4, 5):
            tables[k] = set()
    return tables


_bacc_mod.get_activation_tables = _patched_get_tables

_cache = {}


def _emit_consts(nc, sb, Vp, blocks):
    """Constant tiles + activation warm-up (all overlap the input DMA)."""
    ones = sb.tile([P, 1], F32, name="ones")
    nc.gpsimd.memset(ones[:], 1.0)
    onesr = sb.tile([1, P], F32, name="onesr")
    nc.gpsimd.memset(onesr[:], 1.0)
    itaur = sb.tile([1, P], F32, name="itaur")
    nc.gpsimd.memset(itaur[:], 1.0 / TAU)
    idbs = []
    for i0, rows in blocks:
        idb = sb.tile([P, Vp], F32, name=f"idb{i0}")
        nc.gpsimd.memset(idb[:], 0.0)
        nc.gpsimd.affine_select(
            out=idb[:], in_=idb[:],
            compare_op=mybir.AluOpType.not_equal, fill=1.0, base=i0,
            pattern=[[-1, Vp]], channel_multiplier=1)
        idbs.append(idb)
    ceps = sb.tile([1, 1], F32, name="ceps")
    nc.gpsimd.memset(ceps[:], 1e-30)
    czc = sb.tile([P, 1], F32, name="czc")
    nc.gpsimd.memset(czc[:], float(N - Vp))
    # warm the exp/ln table set; every Act op here lives in that single set
    w1 = sb.tile([1, 1], F32, name="w1")
    nc.scalar.activation(w1, onesr[0:1, 0:1],
                         mybir.ActivationFunctionType.Exp)
    return dict(ones=ones, onesr=onesr, itaur=itaur, idbs=idbs,
                ceps=ceps, czc=czc)


def _emit_loss(nc, sb, psum, ST, Vp, cst, out):
    """Loss from ST (128ch, nm, cb, Vp) f32 segment sums; DMAs to `out`."""
    blocks = [(i0, min(P, Vp - i0)) for i0 in range(0, Vp, P)]
    one1 = cst["onesr"][0:1, 0:1]

    # raw logit matmuls only need ST: issue them first so the in-order
    # PE queue runs them during the norm chain
    ps_Ls = []
    for bi, (i0, rows) in enumerate(blocks):
        ps_L = psum.tile([P, Vp], F32, name=f"ps_L{bi}", tag="ps")
        for cb in range(2):
            nc.tensor.matmul(ps_L[:rows], ST[:, 1, cb, i0:i0 + rows],
                             ST[:, 0, cb, :], start=(cb == 0),
                             stop=(cb == 1))
        ps_Ls.append(ps_L)

    # row norms^2, then 1/norm = exp(-0.5 * ln(nsq + eps))
    sq = sb.tile([P, 2, 2, Vp], F32, name="sq")
    nc.vector.tensor_mul(sq, ST, ST)
    ns_ps = psum.tile([1, 2, Vp], F32, name="ns_ps", tag="ps")
    for cb in range(2):
        nc.tensor.matmul(ns_ps, cst["ones"], sq[:, :, cb, :],
                         start=(cb == 0), stop=(cb == 1))
    lnn = sb.tile([1, 2, Vp], F32, name="lnn")
    nc.scalar.activation(lnn, ns_ps, mybir.ActivationFunctionType.Ln,
                         bias=cst["ceps"])
    inv = sb.tile([1, 2, Vp], F32, name="inv")
    nc.scalar.activation(inv, lnn, mybir.ActivationFunctionType.Exp,
                         scale=-0.5)

    padrow = sb.tile([1, Vp], F32, name="padrow")
    nc.vector.tensor_scalar(padrow, ST[0:1, 1, 0, :], 0.0, None,
                            op0=mybir.AluOpType.not_equal)

    nd = psum.tile([1, 2, len(blocks)], F32, name="nd", tag="ps")
    for bi, (i0, rows) in enumerate(blocks):
        # a_i = inv_k[i0+i] / tau (the 1/tau rides the psum->sbuf copy;
        # PE transpose is pure data movement and ignores identity values)
        a_ps = psum.tile([P, 1], F32, name="a_ps", tag="ps")
        nc.tensor.transpose(a_ps[:rows], inv[:, 1, i0:i0 + rows], one1)
        p_ps = psum.tile([P, 1], F32, name="p_ps", tag="ps")
        nc.tensor.transpose(p_ps[:rows], padrow[:, i0:i0 + rows], one1)
        acol = sb.tile([P, 1], F32, name="acol")
        nc.vector.tensor_scalar_mul(acol[:rows], a_ps[:rows], 1.0 / TAU)
        pcol = sb.tile([P, 1], F32, name="pcol")
        nc.vector.tensor_copy(pcol[:rows], p_ps[:rows])
        # per-col scale Bb[i, j] = inv_q[j] on the (otherwise idle) Pool
        Bb = sb.tile([P, Vp], F32, name="Bb")
        nc.gpsimd.partition_broadcast(Bb[:rows], inv[:, 0, :])
        lg = sb.tile([P, Vp], F32, name="lg")
        nc.vector.scalar_tensor_tensor(lg[:rows], ps_Ls[bi][:rows],
                                       acol[:rows], Bb[:rows],
                                       op0=mybir.AluOpType.mult,
                                       op1=mybir.AluOpType.mult)
        # lse without max subtraction (|logits| <= ~14.3 is exp-safe);
        # Ln bias folds the N - Vp all-zero columns (each exp(0) = 1)
        es = sb.tile([P, Vp], F32, name="es")
        ssum = sb.tile([P, 1], F32, name="ssum")
        nc.scalar.activation(es[:rows], lg[:rows],
                             mybir.ActivationFunctionType.Exp,
                             accum_out=ssum[:rows])
        lse = sb.tile([P, 1], F32, name="lse")
        nc.scalar.activation(lse[:rows], ssum[:rows],
                             mybir.ActivationFunctionType.Ln,
                             bias=cst["czc"][:rows])
        # diag via masked row-sum
        dsel = sb.tile([P, Vp], F32, name="dsel")
        nc.vector.tensor_mul(dsel[:rows], lg[:rows], cst["idbs"][bi][0:rows, :])
        dcol = sb.tile([P, 1], F32, name="dcol")
        nc.vector.tensor_reduce(dcol[:rows], dsel[:rows],
                                axis=mybir.AxisListType.X,
                                op=mybir.AluOpType.add)
        ce = sb.tile([P, 1], F32, name="ce")
        nc.vector.scalar_tensor_tensor(ce[:rows], lse[:rows], dcol[:rows],
                                       pcol[:rows],
                                       op0=mybir.AluOpType.subtract,
                                       op1=mybir.AluOpType.mult)
        nc.tensor.matmul(nd[:, 0, bi:bi + 1], cst["ones"][0:rows], ce[:rows],
                         start=True, stop=True)
        nc.tensor.matmul(nd[:, 1, bi:bi + 1], cst["ones"][0:rows],
                         pcol[:rows], start=True, stop=True)

    num = sb.tile([1, 2], F32, name="num")
    if len(blocks) == 1:
        nc.vector.tensor_copy(num[:, 0:1], nd[:, 0, :])
        nc.vector.tensor_copy(num[:, 1:2], nd[:, 1, :])
    else:
        nc.vector.tensor_reduce(num, nd, axis=mybir.AxisListType.X,
                                op=mybir.AluOpType.add)
    den = sb.tile([1, 1], F32, name="den")
    nc.vector.tensor_scalar_max(den, num[:, 1:2], 1.0)
    rden = sb.tile([1, 1], F32, name="rden")
    nc.vector.reciprocal(rden, den)
    res = sb.tile([1, 1], F32, name="res")
    nc.vector.tensor_mul(res, num[:, 0:1], rden)
    nc.sync.dma_start(out=out[:], in_=res)


def _emit_segsum_matmuls(nc, fin_a, fin_b, psum, T, Vp):
    """Accumulate psT[ch, nm, cb, v] over T tiles. fin_a = [q | comb],
    fin_b = [k]. Groups run back-to-back (interleaved open groups within
    a PSUM bank corrupt results)."""
    COFF = T * 256
    ps = psum.tile([P, 2, 2, Vp], F32, name="ps")
    for nm, src in ((0, fin_a), (1, fin_b)):
        for cb in range(2):
            for t in range(T):
                lo = t * 256 + cb * P
                cmb = fin_a[:, COFF + t * Vp: COFF + (t + 1) * Vp]
                nc.tensor.matmul(ps[:, nm, cb, :], src[:, lo:lo + P],
                                 cmb, start=(t == 0), stop=(t == T - 1))
    return ps


def _build_single(T, Vp):
    """One launch, one core: segment sums + loss."""
    FS = 2 * T * 256 + T * Vp
    KOFF = T * 256 + T * Vp
    nc = bacc.Bacc(None, target_bir_lowering=False, debug=False)
    blocks = [(i0, min(P, Vp - i0)) for i0 in range(0, Vp, P)]
    with tile.TileContext(nc) as tc, ExitStack() as ctx:
        dram = ctx.enter_context(tc.tile_pool(name="dram", bufs=1, space="DRAM"))
        fin = dram.tile([P, FS], FP8, kind="ExternalInput", name="fin",
                        uniquify=False)
        out = dram.tile([1, 1], F32, kind="ExternalOutput", name="loss",
                        uniquify=False)
        sb = ctx.enter_context(tc.tile_pool(name="sb", bufs=1))
        psum = ctx.enter_context(tc.tile_pool(name="psum", bufs=3, space="PSUM"))

        cst = _emit_consts(nc, sb, Vp, blocks)

        fin_a = sb.tile([P, KOFF], FP8, name="fin_a")
        fin_b = sb.tile([P, T * 256], FP8, name="fin_b")
        nc.sync.dma_start(out=fin_a, in_=fin[:, :KOFF])
        nc.sync.dma_start(out=fin_b, in_=fin[:, KOFF:])

        ps = _emit_segsum_matmuls(nc, fin_a, fin_b, psum, T, Vp)
        ST = sb.tile([P, 2, 2, Vp], F32, name="ST")
        nc.vector.tensor_copy(ST, ps)
        _emit_loss(nc, sb, psum, ST, Vp, cst, out)
    nc.compile()
    return nc


def _build_phase1(T, Vp):
    """Fallback launch 1 (8 cores): bf16 partial segment sums."""
    FS = 2 * T * 256 + T * Vp
    KOFF = T * 256 + T * Vp
    nc = bacc.Bacc(None, target_bir_lowering=False, debug=False)
    with tile.TileContext(nc) as tc, ExitStack() as ctx:
        dram = ctx.enter_context(tc.tile_pool(name="dram", bufs=1, space="DRAM"))
        fin = dram.tile([P, FS], FP8, kind="ExternalInput", name="fin",
                        uniquify=False)
        pout = dram.tile([P, 4 * Vp], BF16, kind="ExternalOutput", name="pout",
                         uniquify=False)
        sb = ctx.enter_context(tc.tile_pool(name="sb", bufs=1))
        psum = ctx.enter_context(tc.tile_pool(name="psum", bufs=1, space="PSUM"))

        fin_a = sb.tile([P, KOFF], FP8, name="fin_a")
        fin_b = sb.tile([P, T * 256], FP8, name="fin_b")
        nc.sync.dma_start(out=fin_a, in_=fin[:, :KOFF])
        nc.sync.dma_start(out=fin_b, in_=fin[:, KOFF:])

        ps = _emit_segsum_matmuls(nc, fin_a, fin_b, psum, T, Vp)
        o = sb.tile([P, 2, 2, Vp], BF16, name="o")
        nc.vector.tensor_copy(o, ps)
        nc.sync.dma_start(out=pout[:], in_=o)
    nc.compile()
    return nc


def _build_phase2(Vp):
    """Fallback launch 2 (1 core): combine 8 cores' partials into the loss."""
    nc = bacc.Bacc(None, target_bir_lowering=False, debug=False)
    blocks = [(i0, min(P, Vp - i0)) for i0 in range(0, Vp, P)]
    with tile.TileContext(nc) as tc, ExitStack() as ctx:
        dram = ctx.enter_context(tc.tile_pool(name="dram", bufs=1, space="DRAM"))
        pin = dram.tile([P, 2, 2, Vp, 8], BF16, kind="ExternalInput",
                        name="pin", uniquify=False)
        out = dram.tile([1, 1], F32, kind="ExternalOutput", name="loss",
                        uniquify=False)
        sb = ctx.enter_context(tc.tile_pool(name="sb", bufs=1))
        psum = ctx.enter_context(tc.tile_pool(name="psum", bufs=3, space="PSUM"))

        cst = _emit_consts(nc, sb, Vp, blocks)
        raw = sb.tile([P, 2, 2, Vp, 8], BF16, name="raw")
        nc.sync.dma_start(out=raw, in_=pin[:])
        ST = sb.tile([P, 2, 2, Vp], F32, name="ST")
        nc.vector.tensor_reduce(ST, raw, axis=mybir.AxisListType.X,
                                op=mybir.AluOpType.add)
        _emit_loss(nc, sb, psum, ST, Vp, cst, out)
    nc.compile()
    return nc


def _analyze(mask):
    """Mask-only analysis: valid objects and their (batch, block) work items."""
    mask2 = mask.reshape(B, M, HW)
    mask_flat = mask2.reshape(N, HW)
    ii = np.arange(N)
    comb = mask_flat & mask2[ii % B, ii // B]      # (N, HW)
    vidx = np.nonzero(comb.any(axis=1))[0]
    V = len(vidx)
    if V == 0:
        return comb, vidx, 0, 0, []
    Vp = min(N, max(8, -(-V // 8) * 8))
    combT = comb[vidx].reshape(V, NB_FULL, BS).any(axis=2)  # (V, NB_FULL)
    items = []
    for r in range(B):
        sel = (vidx % B) == r
        if sel.any():
            for t in np.nonzero(combT[sel].any(axis=0))[0]:
                items.append((r, int(t)))
    return comb, vidx, V, Vp, items


def _host_prep(features_q, features_k, pos_region_ranges):
    """Pack mask-selected 16-px feature/comb blocks into contraction tiles."""
    fq = np.asarray(features_q, dtype=np.float32).reshape(B, C, HW)
    fk = np.asarray(features_k, dtype=np.float32).reshape(B, C, HW)
    mask = np.asarray(pos_region_ranges).astype(bool)

    comb, vidx, V, Vp, items = _analyze(mask)
    if V == 0:
        return None
    total_tiles = max(1, -(-len(items) // BPT))
    single = total_tiles <= 128
    ncores = 1 if single else 8
    tiles = [items[j * BPT:(j + 1) * BPT] for j in range(total_tiles)]
    percore = [tiles[c::ncores] for c in range(ncores)]
    T = max(len(pc) for pc in percore)
    FS = 2 * T * 256 + T * Vp

    combV = comb[vidx]                              # (V, HW)
    rsel = [(vidx % B) == r for r in range(B)]
    in_maps = []
    for c in range(ncores):
        fused = np.zeros((P, FS), NP_FP8)
        fq_v = fused[:, :T * 256].reshape(P, T, 256)
        cb_v = fused[:, T * 256:T * 256 + T * Vp].reshape(P, T, Vp)
        fk_v = fused[:, T * 256 + T * Vp:].reshape(P, T, 256)
        for j, blks in enumerate(percore[c]):
            for bi, (r, blk) in enumerate(blks):
                rows = slice(bi * BS, (bi + 1) * BS)
                sl = slice(blk * BS, (blk + 1) * BS)
                fq_v[rows, j, :] = fq[r][:, sl].T.astype(NP_FP8)
                fk_v[rows, j, :] = fk[r][:, sl].T.astype(NP_FP8)
                ct = np.where(rsel[r][:, None], combV[:, sl], False)
                cb_v[rows, j, :V] = ct.T.astype(NP_FP8)
        in_maps.append({"fin": fused})
    return in_maps, V, Vp, T, single


def kernel(features_q, features_k, pos_region_ranges):
    prep = _host_prep(features_q, features_k, pos_region_ranges)
    if prep is None:
        return np.float32(0.0)
    in_maps, V, Vp, T, single = prep

    if single:
        key = ("s", T, Vp)
        if key not in _cache:
            _cache[key] = _build_single(T, Vp)
        nc = _cache[key]
        _cache["single"] = nc
        _cache.pop("p1", None); _cache.pop("p2", None)
        r = run_bass_kernel_spmd(nc, in_maps, core_ids=[0])
        return np.float32(r.results[0]["loss"][0, 0])

    key = ("d", T, Vp)
    if key not in _cache:
        _cache[key] = (_build_phase1(T, Vp), _build_phase2(Vp))
    nc1, nc2 = _cache[key]
    _cache["p1"], _cache["p2"] = nc1, nc2
    _cache.pop("single", None)
    r1 = run_bass_kernel_spmd(nc1, in_maps, core_ids=list(range(8)))
    pin = np.stack([np.asarray(r1.results[s]["pout"]) for s in range(8)],
                   axis=-1).reshape(P, 2, 2, Vp, 8)
    r2 = run_bass_kernel_spmd(nc2, [{"pin": pin}], core_ids=[0])
    return np.float32(r2.results[0]["loss"][0, 0])


# revision 15
# speedup vs baseline: 13.8217x; 1.0096x over previous
"""Trainium2 Bass kernel for nn_ContrastiveLoss (segment_reduce).

The loss depends only on the masked segment means, and a comb row
(mA_i & mB_i) is usually empty or touches a tiny fraction of the image.
Host-side *mask-only* analysis finds
  - the valid objects (nonzero comb rows; the rest enter the loss only as
    exp(0)=1 constants inside the logsumexp, folded into the Ln bias), and
  - the 16-px pixel blocks each valid comb row touches; 8 blocks pack into
    one 128-partition contraction tile.

Fast path (sparse masks, the expected regime): ONE launch on one core —
the few selected feature/comb tiles stream in fp8, PE accumulates the
segment sums psT[ch, v] (output free size = Vp), and the loss chain
(norms via exp(-0.5*ln(nsq)), Vp x Vp logits, lse - diag, masked mean)
runs on the same core. No cross-core combine, no second-launch DMA
latency, full f32 partials.

Fallback (dense masks, > 128 tiles): two launches — 8 cores compute
bf16 partial sums data-parallel over tiles, core 0 combines.

fp8 features keep rel err ~4e-3 (tolerance 2e-2); the /cnt of the
reference cancels inside l2norm; pad == (sk[:,0] != 0) is reproduced
exactly on-chip. All Act ops (Ln, Exp) live in one table set, warmed
during the input DMA. PSUM accumulation groups never interleave within
a bank (that corrupts results).
"""

import numpy as np
from contextlib import ExitStack

import concourse.bass as bass
import concourse.tile as tile
from concourse import bacc, mybir
from concourse.bass_utils import run_bass_kernel_spmd

# Problem constants (hardcoded per task spec)
B, M, C, H, W = 4, 50, 256, 100, 352
HW = H * W                  # 35200
N = B * M                   # 200
TAU = 0.07
P = 128                     # partitions / contraction tile
BS = 4                      # selection block: 4 consecutive pixels
BPT = P // BS               # blocks packed per tile
NB_FULL = HW // BS

F32 = mybir.dt.float32
BF16 = mybir.dt.bfloat16
FP8 = mybir.dt.float8e4
NP_FP8 = mybir.dt.np(FP8)

# Force exp/ln to resolve to the combined "natural_log_exp_and_others" table
# set (index 6): empty the earlier sets we never want so first-match lands on
# sqrt_and_others (3) for sqrt/copy and natural_log_exp_and_others (6) for
# exp+ln. Indices are preserved so act_func_set_id stays aligned.
import concourse.bacc as _bacc_mod
import concourse.hw_specs as _hw_specs
_orig_get_tables = _hw_specs.get_activation_tables


def _patched_get_tables(module_arch):
    tables = dict(_orig_get_tables(module_arch))
    for i, k in enumerate(tables):
        if i in (0, 1, 2, 4, 5):
            tables[k] = set()
    return tables


_bacc_mod.get_activation_tables = _patched_get_tables

_cache = {}


def _emit_consts(nc, sb, Vp, blocks):
    """Constant tiles + activation warm-up (all overlap the input DMA)."""
    ones = sb.tile([P, 1], F32, name="ones")
    nc.gpsimd.memset(ones[:], 1.0)
    onesr = sb.tile([1, P], F32, name="onesr")
    nc.gpsimd.memset(onesr[:], 1.0)
    itaur = sb.tile([1, P], F32, name="itaur")
    nc.gpsimd.memset(itaur[:], 1.0 / TAU)
    idbs = []
    for i0, rows in blocks:
        idb = sb.tile([P, Vp], F32, name=f"idb{i0}")
        nc.gpsimd.memset(idb[:], 0.0)
        nc.gpsimd.affine_select(
            out=idb[:], in_=idb[:],
            compare_op=mybir.AluOpType.not_equal, fill=1.0, base=i0,
            pattern=[[-1, Vp]], channel_multiplier=1)
        idbs.append(idb)
    ceps = sb.tile([1, 1], F32, name="ceps")
    nc.gpsimd.memset(ceps[:], 1e-30)
    czc = sb.tile([P, 1], F32, name="czc")
    nc.gpsimd.memset(czc[:], float(N - Vp))
    # warm the exp/ln table set; every Act op here lives in that single set
    w1 = sb.tile([1, 1], F32, name="w1")
    nc.scalar.activation(w1, onesr[0:1, 0:1],
                         mybir.ActivationFunctionType.Exp)
    return dict(ones=ones, onesr=onesr, itaur=itaur, idbs=idbs,
                ceps=ceps, czc=czc)


def _emit_loss(nc, sb, psum, ST, Vp, cst, out):
    """Loss from ST (128ch, nm, cb, Vp) f32 segment sums; DMAs to `out`."""
    blocks = [(i0, min(P, Vp - i0)) for i0 in range(0, Vp, P)]
    one1 = cst["onesr"][0:1, 0:1]

    # raw logit matmuls only need ST: issue them first so the in-order
    # PE queue runs them during the norm chain
    ps_Ls = []
    for bi, (i0, rows) in enumerate(blocks):
        ps_L = psum.tile([P, Vp], F32, name=f"ps_L{bi}", tag="ps")
        for cb in range(2):
            nc.tensor.matmul(ps_L[:rows], ST[:, 1, cb, i0:i0 + rows],
                             ST[:, 0, cb, :], start=(cb == 0),
                             stop=(cb == 1))
        ps_Ls.append(ps_L)

    # row norms^2, then 1/norm = exp(-0.5 * ln(nsq + eps))
    sq = sb.tile([P, 2, 2, Vp], F32, name="sq")
    nc.vector.tensor_mul(sq, ST, ST)
    ns_ps = psum.tile([1, 2, Vp], F32, name="ns_ps", tag="ps")
    for cb in range(2):
        nc.tensor.matmul(ns_ps, cst["ones"], sq[:, :, cb, :],
                         start=(cb == 0), stop=(cb == 1))
    lnn = sb.tile([1, 2, Vp], F32, name="lnn")
    nc.scalar.activation(lnn, ns_ps, mybir.ActivationFunctionType.Ln,
                         bias=cst["ceps"])
    inv = sb.tile([1, 2, Vp], F32, name="inv")
    nc.scalar.activation(inv, lnn, mybir.ActivationFunctionType.Exp,
                         scale=-0.5)

    padrow = sb.tile([1, Vp], F32, name="padrow")
    nc.vector.tensor_scalar(padrow, ST[0:1, 1, 0, :], 0.0, None,
                            op0=mybir.AluOpType.not_equal)

    nd = psum.tile([1, 2, len(blocks)], F32, name="nd", tag="ps")
    for bi, (i0, rows) in enumerate(blocks):
        # a_i = inv_k[i0+i] / tau (the 1/tau rides the psum->sbuf copy;
        # PE transpose is pure data movement and ignores identity values)
        a_ps = psum.tile([P, 1], F32, name="a_ps", tag="ps")
        nc.tensor.transpose(a_ps[:rows], inv[:, 1, i0:i0 + rows], one1)
        p_ps = psum.tile([P, 1], F32, name="p_ps", tag="ps")
        nc.tensor.transpose(p_ps[:rows], padrow[:, i0:i0 + rows], one1)
        acol = sb.tile([P, 1], F32, name="acol")
        nc.vector.tensor_scalar_mul(acol[:rows], a_ps[:rows], 1.0 / TAU)
        pcol = sb.tile([P, 1], F32, name="pcol")
        nc.vector.tensor_copy(pcol[:rows], p_ps[:rows])
        # per-col scale Bb[i, j] = inv_q[j] on the (otherwise idle) Pool
        Bb = sb.tile([P, Vp], F32, name="Bb")
        nc.gpsimd.partition_broadcast(Bb[:rows], inv[:, 0, :])
        lg = sb.tile([P, Vp], F32, name="lg")
        nc.vector.scalar_tensor_tensor(lg[:rows], ps_Ls[bi][:rows],
                                       acol[:rows], Bb[:rows],
                                       op0=mybir.AluOpType.mult,
                                       op1=mybir.AluOpType.mult)
        # lse without max subtraction (|logits| <= ~14.3 is exp-safe);
        # Ln bias folds the N - Vp all-zero columns (each exp(0) = 1)
        es = sb.tile([P, Vp], F32, name="es")
        ssum = sb.tile([P, 1], F32, name="ssum")
        nc.scalar.activation(es[:rows], lg[:rows],
                             mybir.ActivationFunctionType.Exp,
                             accum_out=ssum[:rows])
        lse = sb.tile([P, 1], F32, name="lse")
        nc.scalar.activation(lse[:rows], ssum[:rows],
                             mybir.ActivationFunctionType.Ln,
                             bias=cst["czc"][:rows])
        # diag via masked row-sum
        dsel = sb.tile([P, Vp], F32, name="dsel")
        nc.vector.tensor_mul(dsel[:rows], lg[:rows], cst["idbs"][bi][0:rows, :])
        dcol = sb.tile([P, 1], F32, name="dcol")
        nc.vector.tensor_reduce(dcol[:rows], dsel[:rows],
                                axis=mybir.AxisListType.X,
                                op=mybir.AluOpType.add)
        ce = sb.tile([P, 1], F32, name="ce")
        nc.vector.scalar_tensor_tensor(ce[:rows], lse[:rows], dcol[:rows],
                                       pcol[:rows],
                                       op0=mybir.AluOpType.subtract,
                                       op1=mybir.AluOpType.mult)
        nc.tensor.matmul(nd[:, 0, bi:bi + 1], cst["ones"][0:rows], ce[:rows],
                         start=True, stop=True)
        nc.tensor.matmul(nd[:, 1, bi:bi + 1], cst["ones"][0:rows],
                         pcol[:rows], start=True, stop=True)

    num = sb.tile([1, 2], F32, name="num")
    if len(blocks) == 1:
        nc.vector.tensor_copy(num[:, 0:1], nd[:, 0, :])
        nc.vector.tensor_copy(num[:, 1:2], nd[:, 1, :])
    else:
        nc.vector.tensor_reduce(num, nd, axis=mybir.AxisListType.X,
                                op=mybir.AluOpType.add)
    den = sb.tile([1, 1], F32, name="den")
    nc.vector.tensor_scalar_max(den, num[:, 1:2], 1.0)
    rden = sb.tile([1, 1], F32, name="rden")
    nc.vector.reciprocal(rden, den)
    res = sb.tile([1, 1], F32, name="res")
    nc.vector.tensor_mul(res, num[:, 0:1], rden)
    nc.sync.dma_start(out=out[:], in_=res)


def _emit_segsum_matmuls(nc, fin_a, fin_b, psum, T, Vp):
    """Accumulate psT[ch, nm, cb, v] over T tiles. fin_a = [q | comb],
    fin_b = [k]. Groups run back-to-back (interleaved open groups within
    a PSUM bank corrupt results)."""
    COFF = T * 256
    ps = psum.tile([P, 2, 2, Vp], F32, name="ps")
    for nm, src in ((0, fin_a), (1, fin_b)):
        for cb in range(2):
            for t in range(T):
                lo = t * 256 + cb * P
                cmb = fin_a[:, COFF + t * Vp: COFF + (t + 1) * Vp]
                nc.tensor.matmul(ps[:, nm, cb, :], src[:, lo:lo + P],
                                 cmb, start=(t == 0), stop=(t == T - 1))
    return ps


def _build_single(T, Vp):
    """One launch, one core: segment sums + loss."""
    FS = 2 * T * 256 + T * Vp
    KOFF = T * 256 + T * Vp
    nc = bacc.Bacc(None, target_bir_lowering=False, debug=False)
    blocks = [(i0, min(P, Vp - i0)) for i0 in range(0, Vp, P)]
    with tile.TileContext(nc) as tc, ExitStack() as ctx:
        dram = ctx.enter_context(tc.tile_pool(name="dram", bufs=1, space="DRAM"))
        fin = dram.tile([P, FS], FP8, kind="ExternalInput", name="fin",
                        uniquify=False)
        out = dram.tile([1, 1], F32, kind="ExternalOutput", name="loss",
                        uniquify=False)
        sb = ctx.enter_context(tc.tile_pool(name="sb", bufs=1))
        psum = ctx.enter_context(tc.tile_pool(name="psum", bufs=3, space="PSUM"))

        cst = _emit_consts(nc, sb, Vp, blocks)

        fin_a = sb.tile([P, KOFF], FP8, name="fin_a")
        fin_b = sb.tile([P, T * 256], FP8, name="fin_b")
        nc.sync.dma_start(out=fin_a, in_=fin[:, :KOFF])
        nc.sync.dma_start(out=fin_b, in_=fin[:, KOFF:])

        ps = _emit_segsum_matmuls(nc, fin_a, fin_b, psum, T, Vp)
        ST = sb.tile([P, 2, 2, Vp], F32, name="ST")
        nc.vector.tensor_copy(ST, ps)
        _emit_loss(nc, sb, psum, ST, Vp, cst, out)
    nc.compile()
    return nc


def _build_phase1(T, Vp):
    """Fallback launch 1 (8 cores): bf16 partial segment sums."""
    FS = 2 * T * 256 + T * Vp
    KOFF = T * 256 + T * Vp
    nc = bacc.Bacc(None, target_bir_lowering=False, debug=False)
    with tile.TileContext(nc) as tc, ExitStack() as ctx:
        dram = ctx.enter_context(tc.tile_pool(name="dram", bufs=1, space="DRAM"))
        fin = dram.tile([P, FS], FP8, kind="ExternalInput", name="fin",
                        uniquify=False)
        pout = dram.tile([P, 4 * Vp], BF16, kind="ExternalOutput", name="pout",
                         uniquify=False)
        sb = ctx.enter_context(tc.tile_pool(name="sb", bufs=1))
        psum = ctx.enter_context(tc.tile_pool(name="psum", bufs=1, space="PSUM"))

        fin_a = sb.tile([P, KOFF], FP8, name="fin_a")
        fin_b = sb.tile([P, T * 256], FP8, name="fin_b")
        nc.sync.dma_start(out=fin_a, in_=fin[:, :KOFF])
        nc.sync.dma_start(out=fin_b, in_=fin[:, KOFF:])

        ps = _emit_segsum_matmuls(nc, fin_a, fin_b, psum, T, Vp)
        o = sb.tile([P, 2, 2, Vp], BF16, name="o")
        nc.vector.tensor_copy(o, ps)
        nc.sync.dma_start(out=pout[:], in_=o)
    nc.compile()
    return nc


def _build_phase2(Vp):
    """Fallback launch 2 (1 core): combine 8 cores' partials into the loss."""
    nc = bacc.Bacc(None, target_bir_lowering=False, debug=False)
    blocks = [(i0, min(P, Vp - i0)) for i0 in range(0, Vp, P)]
    with tile.TileContext(nc) as tc, ExitStack() as ctx:
        dram = ctx.enter_context(tc.tile_pool(name="dram", bufs=1, space="DRAM"))
        pin = dram.tile([P, 2, 2, Vp, 8], BF16, kind="ExternalInput",
                        name="pin", uniquify=False)
        out = dram.tile([1, 1], F32, kind="ExternalOutput", name="loss",
                        uniquify=False)
        sb = ctx.enter_context(tc.tile_pool(name="sb", bufs=1))
        psum = ctx.enter_context(tc.tile_pool(name="psum", bufs=3, space="PSUM"))

        cst = _emit_consts(nc, sb, Vp, blocks)
        raw = sb.tile([P, 2, 2, Vp, 8], BF16, name="raw")
        nc.sync.dma_start(out=raw, in_=pin[:])
        ST = sb.tile([P, 2, 2, Vp], F32, name="ST")
        nc.vector.tensor_reduce(ST, raw, axis=mybir.AxisListType.X,
                                op=mybir.AluOpType.add)
        _emit_loss(nc, sb, psum, ST, Vp, cst, out)
    nc.compile()
    return nc


def _analyze(mask):
    """Mask-only analysis: valid objects and their (batch, block) work items."""
    mask2 = mask.reshape(B, M, HW)
    mask_flat = mask2.reshape(N, HW)
    ii = np.arange(N)
    comb = mask_flat & mask2[ii % B, ii // B]      # (N, HW)
    vidx = np.nonzero(comb.any(axis=1))[0]
    V = len(vidx)
    if V == 0:
        return comb, vidx, 0, 0, []
    Vp = min(N, max(8, -(-V // 8) * 8))
    combT = comb[vidx].reshape(V, NB_FULL, BS).any(axis=2)  # (V, NB_FULL)
    items = []
    for r in range(B):
        sel = (vidx % B) == r
        if sel.any():
            for t in np.nonzero(combT[sel].any(axis=0))[0]:
                items.append((r, int(t)))
    return comb, vidx, V, Vp, items


def _host_prep(features_q, features_k, pos_region_ranges):
    """Pack mask-selected 16-px feature/comb blocks into contraction tiles."""
    fq = np.asarray(features_q, dtype=np.float32).reshape(B, C, HW)
    fk = np.asarray(features_k, dtype=np.float32).reshape(B, C, HW)
    mask = np.asarray(pos_region_ranges).astype(bool)

    comb, vidx, V, Vp, items = _analyze(mask)
    if V == 0:
        return None
    total_tiles = max(1, -(-len(items) // BPT))
    single = total_tiles <= 128
    ncores = 1 if single else 8
    tiles = [items[j * BPT:(j + 1) * BPT] for j in range(total_tiles)]
    percore = [tiles[c::ncores] for c in range(ncores)]
    T = max(len(pc) for pc in percore)
    FS = 2 * T * 256 + T * Vp

    combV = comb[vidx]                              # (V, HW)
    rsel = [(vidx % B) == r for r in range(B)]
    in_maps = []
    for c in range(ncores):
        fused = np.zeros((P, FS), NP_FP8)
        fq_v = fused[:, :T * 256].reshape(P, T, 256)
        cb_v = fused[:, T * 256:T * 256 + T * Vp].reshape(P, T, Vp)
        fk_v = fused[:, T * 256 + T * Vp:].reshape(P, T, 256)
        for j, blks in enumerate(percore[c]):
            for bi, (r, blk) in enumerate(blks):
                rows = slice(bi * BS, (bi + 1) * BS)
                sl = slice(blk * BS, (blk + 1) * BS)
                fq_v[rows, j, :] = fq[r][:, sl].T.astype(NP_FP8)
                fk_v[rows, j, :] = fk[r][:, sl].T.astype(NP_FP8)
                ct = np.where(rsel[r][:, None], combV[:, sl], False)
                cb_v[rows, j, :V] = ct.T.astype(NP_FP8)
        in_maps.append({"fin": fused})
    return in_maps, V, Vp, T, single


def kernel(features_q, features_k, pos_region_ranges):
    prep = _host_prep(features_q, features_k, pos_region_ranges)
    if prep is None:
        return np.float32(0.0)
    in_maps, V, Vp, T, single = prep

    if single:
        key = ("s", T, Vp)
        if key not in _cache:
            _cache[key] = _build_single(T, Vp)
        nc = _cache[key]
        _cache["single"] = nc
        _cache.pop("p1", None); _cache.pop("p2", None)
        r = run_bass_kernel_spmd(nc, in_maps, core_ids=[0])
        return np.float32(r.results[0]["loss"][0, 0])

    key = ("d", T, Vp)
    if key not in _cache:
        _cache[key] = (_build_phase1(T, Vp), _build_phase2(Vp))
    nc1, nc2 = _cache[key]
    _cache["p1"], _cache["p2"] = nc1, nc2
    _cache.pop("single", None)
    r1 = run_bass_kernel_spmd(nc1, in_maps, core_ids=list(range(8)))
    pin = np.stack([np.asarray(r1.results[s]["pout"]) for s in range(8)],
                   axis=-1).reshape(P, 2, 2, Vp, 8)
    r2 = run_bass_kernel_spmd(nc2, [{"pin": pin}], core_ids=[0])
    return np.float32(r2.results[0]["loss"][0, 0])
